# revision 1
# baseline (speedup 1.0000x reference)
"""Trainium2 Bass kernel for nn_BoxesFromMasks (per-frame segment bounding boxes).

Algorithm (per core, data-parallel over frames):
  For each frame, build per-pixel one-hot bitmasks of the instance id using an
  exponent-bit trick (int ops construct the bit pattern of float 2^k, an ACT
  copy casts float->uint32 which truncates out-of-range ids to 0):
    lo plane: id s in [0,32)  -> bit (31-s)
    hi plane: id s in [32,64) -> bit (s-32)
  Row masks:  OR-reduce each 128-row chunk along the free (column) axis.
  Col masks:  OR-accumulate chunks into a per-column accumulator, then
              DMA-transpose (as uint16) and OR-reduce along rows.
  Extraction: expand mask bits per id with constant tables, select coordinate
              values, min/max reduce, and partition-fold to one partition.
"""

import numpy as np

_T, _H, _W, _N = 16, 1024, 2048, 64
_NCORES = 8

_BUILD_CACHE = {}


def _build_program(TL, H, W, split_waits=True, reps=1, dbg=False):
    from contextlib import ExitStack

    import concourse.bass as bass
    import concourse.tile as tile
    import concourse.mybir as mybir
    from concourse.alu_op_type import AluOpType as Op

    f32 = mybir.dt.float32
    i32 = mybir.dt.int32
    u32 = mybir.dt.uint32
    u16 = mybir.dt.uint16
    Copy = mybir.ActivationFunctionType.Copy
    X = mybir.AxisListType.X

    P = 128
    CH = H // P                   # row chunks per frame
    UC = 2 * W                    # u16 columns per plane
    KT = 8 if UC % (128 * 8) == 0 else UC // 128   # transpose DMA splits
    SPLIT = UC // KT              # u16 cols per transpose call
    MPER = SPLIT // 128           # mid-dim blocks per call
    B = UC // 128                 # total transposed blocks (unused)
    BH = W // 128                 # blocks per halfword table
    BIG = 0x7FFF                  # absent sentinel (fits i16, fp32-exact)
    BIG16 = 0x7FFF

    # ---- constant tables ----
    pp = np.arange(P)
    yv = (np.arange(CH)[None, :] * P + pp[:, None]).astype(np.int64)    # [P, CH]
    bb = np.arange(B)
    xv = ((SPLIT // 2) * (bb[None, :] // MPER) + 64 * (bb[None, :] % MPER)
          + (pp[:, None] % 64)).astype(np.int64)                        # [P, B]
    # select-value scalars, fp32 (tensor_scalar AP scalars must be fp32;
    # every value is < 2^15 so fp32 arithmetic on them is exact)
    tables = {
        "ymB": (yv - BIG).astype(np.float32),
        "yp1": (yv + 1).astype(np.float32),
        "xmB": (xv - BIG16).astype(np.float32),
        "xp1": (xv + 1).astype(np.float32),
    }

    nc = bass.Bass()
    seg_in = nc.dram_tensor("seg", [TL, H, W], i32, kind="ExternalInput")
    boxes_out = nc.dram_tensor("boxes", [TL, 64, 4], f32, kind="ExternalOutput")

    cmbounce = nc.dram_tensor("cmbounce", [TL, P, 2, B], u16)
    d_ymB, d_yp1, d_xmB, d_xp1 = (
        nc.dram_tensor(n, list(tables[n].shape), f32, kind="ExternalInput")
        for n in ["ymB", "yp1", "xmB", "xp1"])

    if dbg:
        dbg_rmask = nc.dram_tensor("dbg_rmask", [P, TL, 2, CH], u32,
                                   kind="ExternalOutput")
        dbg_cmask = nc.dram_tensor("dbg_cmask", [P, TL, 2, 2, B], u16,
                                   kind="ExternalOutput")
        dbg_E32 = nc.dram_tensor("dbg_E32", [P, 2, 32, TL, CH], i32,
                                 kind="ExternalOutput")
        dbg_E16 = nc.dram_tensor("dbg_E16", [P, 2, 32, TL, B], mybir.dt.int16,
                                 kind="ExternalOutput")
        dbg_SR = nc.dram_tensor("dbg_SR", [P, 4], mybir.dt.int16,
                                kind="ExternalOutput")

    with tile.TileContext(nc) as tc, ExitStack() as ctx:
        constp = ctx.enter_context(tc.tile_pool(name="consts", bufs=1))
        segp = ctx.enter_context(tc.tile_pool(name="segp", bufs=2))
        ep = ctx.enter_context(tc.tile_pool(name="ep", bufs=3))
        accp = ctx.enter_context(tc.tile_pool(name="accp", bufs=2))
        accTp = ctx.enter_context(tc.tile_pool(name="accTp", bufs=1))
        maskp = ctx.enter_context(tc.tile_pool(name="maskp", bufs=1))
        xp = ctx.enter_context(tc.tile_pool(name="xp", bufs=2))
        trp = ctx.enter_context(tc.tile_pool(name="trp", bufs=2))
        smallp = ctx.enter_context(tc.tile_pool(name="smallp", bufs=1))

        c_ymB = constp.tile([P, CH], f32)
        nc.sync.dma_start(c_ymB[:], d_ymB[:])
        c_yp1 = constp.tile([P, CH], f32)
        nc.sync.dma_start(c_yp1[:], d_yp1[:])
        c_xmB = constp.tile([P, B], f32)
        nc.sync.dma_start(c_xmB[:], d_xmB[:])
        c_xp1 = constp.tile([P, B], f32)
        nc.sync.dma_start(c_xp1[:], d_xp1[:])

        # body repeated `reps` times (identical output; used for wall-clock
        # device-time measurement: (wall(R) - wall(1)) / (R - 1))
        for _rep in range(reps):
            rmask16 = maskp.tile([P, TL, 2, CH, 16], u32, tag="rmask16")
            cmask = maskp.tile([P, TL, 2, 2, B], u16, tag="cmask")

            # ================= main loop =================
            for f in range(TL):
                acc = accp.tile([P, 2, W], u32)
                prev_u = None
                for c in range(CH):
                    s = segp.tile([P, W], i32)
                    for k in range(8):
                        nc.sync.dma_start(
                            s[16 * k:16 * (k + 1), :],
                            seg_in[f, c * P + 16 * k:c * P + 16 * (k + 1), :])

                    e = ep.tile([P, 2, W], i32)
                    # lo: bitpattern of 2^(31-s) = (158-s)<<23 ; hi: 2^(s-32) = (s+95)<<23
                    nc.scalar.activation(e[:, 0, :], s[:], Copy,
                                         bias=1325400064.0, scale=-8388608.0)
                    nc.gpsimd.tensor_scalar(e[:, 1, :], s[:], 8388608, 796917760,
                                            Op.mult, Op.add)
                    u = e[:].bitcast(u32)  # in-place cast target
                    nc.scalar.activation(u, e[:].bitcast(f32), Copy)

                    # column accumulate (DVE; only DVE has integer bitwise ops)
                    if c == 0:
                        prev_u = u
                    elif c == 1:
                        nc.vector.tensor_tensor(acc[:], u, prev_u, Op.bitwise_or)
                    else:
                        nc.vector.tensor_tensor(acc[:], u, acc[:], Op.bitwise_or)

                    # row masks: OR-tree along columns (DVE). In place, except
                    # chunk 0 whose u must stay intact for the c==1 accumulate.
                    if c == 0:
                        tr0 = trp.tile([P, 2, W // 2], u32, tag="tr0")
                        base = tr0[:]
                    else:
                        base = e[:, :, 0:W // 2].bitcast(u32)
                    w = W // 2
                    nc.vector.tensor_tensor(base[:, :, 0:w], u[:, :, 0:w],
                                            u[:, :, w:2 * w], Op.bitwise_or)
                    w //= 2
                    while w >= 16:
                        nc.vector.tensor_tensor(base[:, :, 0:w], base[:, :, 0:w],
                                                base[:, :, w:2 * w], Op.bitwise_or)
                        w //= 2
                    # leftovers [P, 2, 16] -> rmask16 wide buffer; folded later
                    nc.vector.tensor_copy(rmask16[:, f, :, c, :], base[:, :, 0:16])

                # ---- transpose acc as u16 and OR-reduce rows; then
                # parity-sort partitions so halfword planes are contiguous:
                # u16col = k*SPLIT + 128*m + p, so halfword h = p & 1.
                accT = accTp.tile([P, 2, B, 128], u16)
                for pl in range(2):
                    a16 = acc[:, pl, :].bitcast(u16)   # [P, UC]
                    for k in range(KT):
                        nc.sync.dma_start(accT[:, pl, k * MPER:(k + 1) * MPER, :],
                                          a16[:, k * SPLIT:(k + 1) * SPLIT],
                                          transpose=True)
                w = 64
                while w >= 1:
                    nc.vector.tensor_tensor(accT[:, :, :, 0:w], accT[:, :, :, 0:w],
                                            accT[:, :, :, w:2 * w], Op.bitwise_or)
                    w //= 2
                # cmask[q, f, pl, h, b]: q<64 <-> p=2q (h=0), q>=64 <-> p=2q+1
                # partition parity sort via a small DRAM bounce
                cmtmp = smallp.tile([P, 2, B], u16, tag="cmtmp")
                nc.vector.tensor_copy(cmtmp[:], accT[:, :, :, 0])
                nc.sync.dma_start(cmbounce[f], cmtmp[:])
                cb = cmbounce[f].rearrange("(q two) a b -> q two a b", two=2)
                for h in range(2):
                    nc.sync.dma_start(cmask[64 * h:64 * (h + 1), f, :, h, :],
                                      cb[:, h, :, :])

            # ================= extraction =================
            i16 = mybir.dt.int16
            assert 2 * 32 * TL == 128  # per-stat slot block == one transpose column set

            # ISA APs allow at most 3 free dims: expand per plane, then flatten
            # (pl, s', f) -> one 128-wide dim for the value-select and reduce.
            def flat1(t):
                return t[:].rearrange("p a b c d -> p (a b c d)")

            def flat3(t):
                return t[:].rearrange("p a b c d -> p (a b c) d")

            # fold rowmask leftovers [.., 16] -> [.., 1]
            rmf = rmask16[:].rearrange("p a b c w -> p (a b c) w")
            w = 8
            while w >= 1:
                nc.vector.tensor_tensor(rmf[:, :, 0:w], rmf[:, :, 0:w],
                                        rmf[:, :, w:2 * w], Op.bitwise_or)
                w //= 2

            if dbg:
                nc.sync.dma_start(dbg_rmask[:], rmask16[:, :, :, :, 0])
                nc.sync.dma_start(dbg_cmask[:], cmask[:])

            # ---- row side: ymin / ymax ----
            # E = (mask >> bit) & 1  (one op per id slot; int immediates)
            E32 = xp.tile([P, 2, 32, TL, CH], i32, tag="xE")
            for pl in range(2):
                rm_v = rmask16[:, :, pl, :, 0]            # [P, TL, CH]
                for sp in range(32):
                    bit = (31 - sp) if pl == 0 else sp
                    nc.vector.tensor_scalar(
                        E32[:, pl, sp], rm_v.bitcast(i32), bit, 1,
                        Op.logical_shift_right, Op.bitwise_and)

            if dbg:
                nc.sync.dma_start(dbg_E32[:], E32[:])

            # cmin = E*(v-BIG) + BIG in {v, BIG}; cmax = E*(v+1) in {v+1, 0}
            cmin32 = xp.tile([P, 2, 32, TL, CH], i32, tag="xc")
            cmax32 = xp.tile([P, 2, 32, TL, CH], i32, tag="xc")
            for c in range(CH):
                nc.scalar.activation(
                    cmin32[:, :, :, :, c], E32[:, :, :, :, c], Copy,
                    scale=c_ymB[:, c].unsqueeze(1), bias=float(BIG))
                nc.scalar.activation(
                    cmax32[:, :, :, :, c], E32[:, :, :, :, c], Copy,
                    scale=c_yp1[:, c].unsqueeze(1), bias=0.0)

            rmin = smallp.tile([P, 2, 32, TL], i32)
            rmax = smallp.tile([P, 2, 32, TL], i32)
            rmin_f = rmin[:].rearrange("p a b f -> p (a b f)")
            rmax_f = rmax[:].rearrange("p a b f -> p (a b f)")
            nc.vector.tensor_reduce(rmin_f, flat3(cmin32), axis=X, op=Op.min)
            nc.vector.tensor_reduce(rmax_f, flat3(cmax32), axis=X, op=Op.max)

            # ---- col side: xmin / xmax ----
            # each slot's bits live in one parity-half of the partitions;
            # zero the rest so they stay neutral through the select.
            E16 = xp.tile([P, 2, 32, TL, B], i16, tag="xE")
            nc.gpsimd.memset(E16[:], 0)
            for pl in range(2):
                for sp in range(32):
                    bit = (31 - sp) if pl == 0 else sp
                    h_req, inbit = bit >> 4, bit & 15
                    q0 = 64 * h_req
                    cm_v = cmask[q0:q0 + 64, :, pl, h_req, :]   # [64, TL, B]
                    nc.vector.tensor_scalar(
                        E16[q0:q0 + 64, pl, sp], cm_v.bitcast(i16), inbit, 1,
                        Op.logical_shift_right, Op.bitwise_and)

            if dbg:
                nc.sync.dma_start(dbg_E16[:], E16[:])

            cmin16 = xp.tile([P, 2, 32, TL, B], i16, tag="xc")
            cmax16 = xp.tile([P, 2, 32, TL, B], i16, tag="xc")
            for b in range(B):
                nc.scalar.activation(
                    cmin16[:, :, :, :, b], E16[:, :, :, :, b], Copy,
                    scale=c_xmB[:, b].unsqueeze(1), bias=float(BIG16))
                nc.scalar.activation(
                    cmax16[:, :, :, :, b], E16[:, :, :, :, b], Copy,
                    scale=c_xp1[:, b].unsqueeze(1), bias=0.0)

            # combined signed stat tile: S[p, k, pl, s', f], k: 0=-xmin 1=-ymin
            # 2=xmax+1 3=ymax+1 (max-fold works for all four)
            S = smallp.tile([P, 4, 2, 32, TL], i16)

            def srow(k, dt=None):
                ap = S[:, k].rearrange("p a b f -> p (a b f)")
                return ap.bitcast(dt) if dt is not None else ap

            nc.vector.tensor_reduce(srow(0), flat3(cmin16), axis=X, op=Op.min)
            nc.vector.tensor_scalar(srow(0), srow(0), -1, 0, Op.mult, Op.add)
            nc.vector.tensor_copy(srow(1), rmin_f)
            nc.vector.tensor_scalar(srow(1), srow(1), -1, 0, Op.mult, Op.add)
            nc.vector.tensor_reduce(srow(2), flat3(cmax16), axis=X, op=Op.max)
            nc.vector.tensor_copy(srow(3), rmax_f)

            # partition fold via u16 DMA transpose + X-reduce over source partitions
            S2 = S[:].rearrange("p k a b f -> p (k a b f)")   # [128, 512]
            ST = smallp.tile([P, 4, 128], i16)
            for m in range(4):
                nc.sync.dma_start(ST[:, m, :], S2[:, 128 * m:128 * (m + 1)],
                                  transpose=True)
            SR = smallp.tile([P, 4], i16)
            nc.vector.tensor_reduce(SR[:], ST[:], axis=X, op=Op.max)

            if dbg:
                nc.sync.dma_start(dbg_SR[:], SR[:])

            # finalize: V[p, k] with p = (pl*32+s')*TL + f
            V = smallp.tile([P, 4], i32)
            nc.vector.tensor_copy(V[:], SR[:])
            nc.vector.tensor_scalar(V[:, 0:2], V[:, 0:2], -1, 0, Op.mult, Op.add)
            nc.vector.tensor_scalar(V[:, 2:4], V[:, 2:4], 1, 0, Op.subtract, Op.add)
            BOF = smallp.tile([P, 4], f32)
            fix = smallp.tile([P, 4], f32)
            nc.vector.tensor_copy(BOF[:], V[:])
            # empty segments (in f32, so the sums round exactly to +/-2^31):
            # mins 32767 -> 2147483648.0, maxes -1 -> -2147483648.0
            nc.vector.tensor_scalar(fix[:, 0:2], BOF[:, 0:2], 32767.0, 2147450880.0,
                                    Op.is_equal, Op.mult)
            nc.vector.tensor_scalar(fix[:, 2:4], BOF[:, 2:4], -1.0, -2147483647.0,
                                    Op.is_equal, Op.mult)
            nc.vector.tensor_tensor(BOF[:], BOF[:], fix[:], Op.add)

            # boxes[f, n, k] <- BOF[n*TL + f, k]
            nc.sync.dma_start(boxes_out[:].transpose([1, 0, 2]), BOF[:])

    nc.finalize()
    if split_waits:
        _split_excess_waits(nc, mybir)
    return nc, tables


def _split_excess_waits(nc, mybir):
    """Hoist extra sem waits onto preceding NoOps.

    This walrus build rejects instructions carrying more sync-wait
    conditions than their ISA encoding holds (1 for TPB_CTRL ops and for
    Pool/core_v2 compute ops; 2 elsewhere, conservatively). Semantics are
    identical with the waits split onto dedicated NoOps just before the
    instruction.
    """
    ctrl = {"Drain", "NoOp", "Nop", "EventSemaphore", "AllEngineBarrier"}
    n_split = 0
    for f in nc.m.functions:
        for bb in f.blocks:
            newl = []
            for ins in bb.instructions:
                si = ins.sync_info
                max_waits = 1
                if si and si.on_wait and len(si.on_wait) > max_waits:
                    waits = list(si.on_wait)
                    for j, w in enumerate(waits[max_waits:]):
                        nop = mybir.InstNoOp(
                            name=f"{ins.name}-w{j}", ins=[], outs=[],
                            engine=ins.engine,
                            sync_info=mybir.SyncInfo(on_wait=[w], on_update=[]))
                        newl.append(nop)
                        n_split += 1
                    ins.sync_info = mybir.SyncInfo(on_wait=waits[:max_waits],
                                                   on_update=si.on_update)
                newl.append(ins)
            bb.instructions = newl
    return n_split


def _get_program(TL, H, W, reps=1):
    key = (TL, H, W, reps)
    if key not in _BUILD_CACHE:
        _BUILD_CACHE[key] = _build_program(TL, H, W, reps=reps)
    return _BUILD_CACHE[key]


def kernel(segmentation, num_instances=None, **_ignored):
    from concourse.bass_utils import run_bass_kernel_spmd

    seg = np.asarray(segmentation)
    T, H, W = seg.shape
    assert T % _NCORES == 0
    TL = T // _NCORES
    nc, tables = _get_program(TL, H, W)

    seg = np.ascontiguousarray(seg, dtype=np.int32)
    in_maps = [{"seg": seg[i * TL:(i + 1) * TL], **tables}
               for i in range(_NCORES)]
    res = run_bass_kernel_spmd(nc, in_maps, list(range(_NCORES)))
    out = np.concatenate([res.results[i]["boxes"] for i in range(_NCORES)], axis=0)
    return out.astype(np.float32)



# revision 8
# speedup vs baseline: 1.2944x; 1.2944x over previous
"""Trainium2 Bass kernel for nn_BoxesFromMasks (per-frame segment bounding boxes).

Algorithm (per core, data-parallel over frames, TL=2 frames/core):
  Build per-pixel 64-bit one-hot bitmasks (2 u32 planes) of the instance id via
  the exponent-bit trick (ACT builds the f32 bit pattern of 2^k as an int, a
  second ACT converts value->u32, truncating out-of-range ids to 0):
    lo plane: id s in [0,32)  -> bit (31-s)
    hi plane: id s in [32,64) -> bit (s-32)
  Row masks:  OR-tree each 128-row chunk along columns (DVE), 16-wide leftovers
              folded once at extraction time.
  Col masks:  OR-accumulate chunks into acc[128,2,W]; pre-fold partitions
              128->64; DMA-transpose (u16); OR-tree the 64 contributors.
  Extraction (batched, no DRAM bounce): 16 u16 shift ops expand bits to
  E-tables, constant value-tables select coordinates via i16 mult/add, strided
  tensor_reduce min/max, one 3x128 transpose fold, and negative-stride output
  DMAs undo the bit-order permutation.
"""

import numpy as np

_T, _H, _W, _N = 16, 1024, 2048, 64
_NCORES = 8

_BUILD_CACHE = {}


def _build_program(TL, H, W, split_waits=True, reps=1):
    from contextlib import ExitStack

    import bass_rust
    import concourse.bass as bass
    import concourse.tile as tile
    import concourse.mybir as mybir
    from concourse.alu_op_type import AluOpType as Op

    f32 = mybir.dt.float32
    i32 = mybir.dt.int32
    u32 = mybir.dt.uint32
    u16 = mybir.dt.uint16
    i16 = mybir.dt.int16
    Copy = mybir.ActivationFunctionType.Copy
    X = mybir.AxisListType.X

    P = 128
    CH = H // P                   # row chunks per frame (8)
    KT = 4                        # transpose calls per frame (each 2048 u16 cols)
    MPER = 16                     # 128-col blocks per transpose call
    NSEG = 2                      # seg DMA splits per chunk
    BIG = 32767
    assert TL == 2 and CH == 8 and W == 2048

    # ---- constant value tables (i16) ----
    pp = np.arange(P)
    # Y: value v(p, c) = 128c + p ; table shape [P, 64(pl f j), CH, 2h]
    yv = (128 * np.arange(CH)[None, :] + pp[:, None]).astype(np.int64)   # [P, CH]
    ty_mb = np.broadcast_to((yv - BIG)[:, None, :, None],
                            (P, 64, CH, 2)).astype(np.int16)
    ty_p1 = np.broadcast_to((yv + 1)[:, None, :, None],
                            (P, 64, CH, 2)).astype(np.int16)
    # X: value v(q, klo, m) = klo*1024 + 64m + (q>>1) ; table [P, 64(pl f j), 32(klo m)]
    klo = np.arange(2)
    mm = np.arange(MPER)
    xv = ((klo[:, None] * 1024 + 64 * mm[None, :]).reshape(-1)[None, :]
          + (pp[:, None] // 2)).astype(np.int64)                         # [P, 32]
    tx_mb = np.broadcast_to((xv - BIG)[:, None, :], (P, 64, 32)).astype(np.int16)
    tx_p1 = np.broadcast_to((xv + 1)[:, None, :], (P, 64, 32)).astype(np.int16)

    tables = {"ty_mb": ty_mb, "ty_p1": ty_p1, "tx_mb": tx_mb, "tx_p1": tx_p1}

    nc = bass.Bass()
    seg_in = nc.dram_tensor("seg", [TL, H, W], i32, kind="ExternalInput")
    boxes_out = nc.dram_tensor("boxes", [TL, 64, 4], f32, kind="ExternalOutput")
    d_tabs = {n: nc.dram_tensor(n, list(t.shape), i16, kind="ExternalInput")
              for n, t in tables.items()}

    def dram_ap(t, offset_elems, dims):
        """Manual DRAM AP: dims = [(stride_elems, count), ...]."""
        a2 = t[:].copy()
        a2.offset = offset_elems
        a2.ap = bass_rust.VecI64Pair([[s, n] for s, n in dims])
        return a2

    with tile.TileContext(nc) as tc, ExitStack() as ctx:
        constp = ctx.enter_context(tc.tile_pool(name="consts", bufs=1))
        segp = ctx.enter_context(tc.tile_pool(name="segp", bufs=3))
        ep = ctx.enter_context(tc.tile_pool(name="ep", bufs=2))
        accp = ctx.enter_context(tc.tile_pool(name="accp", bufs=2))
        accTp = ctx.enter_context(tc.tile_pool(name="accTp", bufs=2))
        rmp = ctx.enter_context(tc.tile_pool(name="rmp", bufs=1))
        trp = ctx.enter_context(tc.tile_pool(name="trp", bufs=2))
        xp = ctx.enter_context(tc.tile_pool(name="xp", bufs=1))
        smallp = ctx.enter_context(tc.tile_pool(name="smallp", bufs=1))

        c_ty_mb = constp.tile([P, 64, CH, 2], i16)
        c_ty_p1 = constp.tile([P, 64, CH, 2], i16)
        c_tx_mb = constp.tile([P, 64, 32], i16)
        c_tx_p1 = constp.tile([P, 64, 32], i16)
        for t, n in [(c_ty_mb, "ty_mb"), (c_ty_p1, "ty_p1"),
                     (c_tx_mb, "tx_mb"), (c_tx_p1, "tx_p1")]:
            nc.sync.dma_start(t[:], d_tabs[n][:])

        for _rep in range(reps):
            # rmask16: [p, pl, f, c, 16] u32 (pl-major for contiguous planes)
            rmask16 = rmp.tile([P, 2, TL, CH, 16], u32, tag="rmask16")
            # CMX: [q, pl, f, klo, m] u16 (compacted column masks)
            CMX = xp.tile([P, 2, TL, 2, MPER], u16, tag="cmx")

            # ================= main loop =================
            for f in range(TL):
                acc = accp.tile([P, 2, W], u32)
                prev_u = None
                for c in range(CH):
                    s = segp.tile([P, W], i32)
                    rows = P // NSEG
                    for k in range(NSEG):
                        nc.sync.dma_start(
                            s[rows * k:rows * (k + 1), :],
                            seg_in[f, c * P + rows * k:c * P + rows * (k + 1), :])

                    e = ep.tile([P, 2, W], i32)
                    # lo: bitpattern of 2^(31-s) = (158-s)<<23 ; hi: 2^(s-32) = (s+95)<<23
                    nc.scalar.activation(e[:, 0, :], s[:], Copy,
                                         bias=1325400064.0, scale=-8388608.0)
                    nc.gpsimd.tensor_scalar(e[:, 1, :], s[:], 8388608, 796917760,
                                            Op.mult, Op.add)
                    u = e[:].bitcast(u32)  # in-place cast target
                    nc.scalar.activation(u, e[:].bitcast(f32), Copy)

                    # column accumulate (DVE; only DVE has integer bitwise ops)
                    if c == 0:
                        prev_u = u
                    elif c == 1:
                        nc.vector.tensor_tensor(acc[:], u, prev_u, Op.bitwise_or)
                    else:
                        nc.vector.tensor_tensor(acc[:], u, acc[:], Op.bitwise_or)

                    # row masks: OR-tree along columns (DVE). In place, except
                    # chunk 0 whose u must stay intact for the c==1 accumulate.
                    if c == 0:
                        tr0 = trp.tile([P, 2, W // 2], u32, tag="tr0")
                        base = tr0[:]
                    else:
                        base = e[:, :, 0:W // 2].bitcast(u32)
                    w = W // 2
                    nc.vector.tensor_tensor(base[:, :, 0:w], u[:, :, 0:w],
                                            u[:, :, w:2 * w], Op.bitwise_or)
                    w //= 2
                    while w >= 16:
                        nc.vector.tensor_tensor(base[:, :, 0:w], base[:, :, 0:w],
                                                base[:, :, w:2 * w], Op.bitwise_or)
                        w //= 2
                    nc.vector.tensor_copy(rmask16[:, :, f, c, :], base[:, :, 0:16])

                # ---- frame tail: transpose (u16), fold the 128 contributors
                accT = accTp.tile([P, KT, MPER, P], u16, tag="accT")
                a16 = acc[:].bitcast(u16).rearrange("p a b -> p (a b)")
                for k in range(KT):
                    nc.sync.dma_start(accT[:, k],
                                      a16[:, 2048 * k:2048 * (k + 1)],
                                      transpose=True)
                w = 64
                while w >= 1:
                    nc.vector.tensor_tensor(accT[:, :, :, 0:w],
                                            accT[:, :, :, 0:w],
                                            accT[:, :, :, w:2 * w],
                                            Op.bitwise_or)
                    w //= 2
                # compact: CMX[q, pl, f, klo, m] <- accT[q, (pl,klo), m, 0]
                nc.vector.tensor_copy(
                    CMX[:, :, f, :, :],
                    accT[:, :, :, 0].rearrange("q (pl klo) m -> q pl klo m",
                                               pl=2, klo=2))

            # ================= extraction =================
            # fold rowmask leftovers [..., 16] -> [..., 1]
            rmf = rmask16[:].rearrange("p pl f c w -> p (pl f c) w")
            w = 8
            while w >= 1:
                nc.vector.tensor_tensor(rmf[:, :, 0:w], rmf[:, :, 0:w],
                                        rmf[:, :, w:2 * w], Op.bitwise_or)
                w //= 2

            # --- Y side ---
            # rm u16 view: [p, pl, f, c, h]  (h = u16 half; bit b32 = 16h + j)
            rmv = rmask16[:].bitcast(u16)[:, :, :, :, 0:2]
            rm_e = rmv.rearrange("p pl f c h -> p (pl f) c h")
            Ey = xp.tile([P, 2, TL, 16, CH, 2], i16, tag="ey")
            for j in range(16):
                nc.vector.tensor_scalar(
                    Ey[:, :, :, j].rearrange("p pl f c h -> p (pl f) c h").bitcast(u16),
                    rm_e, j, 1, Op.logical_shift_right, Op.bitwise_and)

            ey_flat = Ey[:].rearrange("p pl f j c h -> p (pl f j) c h")
            CY = xp.tile([P, 64, CH, 2], i16, tag="cy")
            S = smallp.tile([P, 384], i16)
            # Y block layout: col = t*128 + h*64 + (pl f j)  (h-major for output DMAs)
            Sy = S[:, 0:256].rearrange("p (t h a) -> p t h a", t=2, h=2, a=64)
            # ymin: min over c of E*(v-BIG)+BIG
            nc.vector.tensor_tensor(CY[:], ey_flat, c_ty_mb[:], Op.mult)
            nc.vector.tensor_scalar(CY[:], CY[:], BIG, None, Op.add)
            nc.vector.tensor_reduce(Sy[:, 0], CY[:].rearrange("p a c h -> p h a c"),
                                    axis=X, op=Op.min)
            # ymax(+1): max over c of E*(v+1)
            nc.vector.tensor_tensor(CY[:], ey_flat, c_ty_p1[:], Op.mult)
            nc.vector.tensor_reduce(Sy[:, 1], CY[:].rearrange("p a c h -> p h a c"),
                                    axis=X, op=Op.max)

            # --- X side ---
            cmx_flat = CMX[:].rearrange("q pl f klo m -> q (pl f) (klo m)")
            Ex = xp.tile([P, 4, 16, 32], i16, tag="ex")
            for j in range(16):
                nc.vector.tensor_scalar(Ex[:, :, j].bitcast(u16), cmx_flat,
                                        j, 1, Op.logical_shift_right, Op.bitwise_and)
            ex_flat = Ex[:].rearrange("q a j km -> q (a j) km")
            CXt = xp.tile([P, 64, 32], i16, tag="cx")
            nc.vector.tensor_tensor(CXt[:], ex_flat, c_tx_mb[:], Op.mult)
            nc.vector.tensor_scalar(CXt[:], CXt[:], BIG, None, Op.add)
            nc.vector.tensor_reduce(S[:, 256:320], CXt[:], axis=X, op=Op.min)
            nc.vector.tensor_tensor(CXt[:], ex_flat, c_tx_p1[:], Op.mult)
            nc.vector.tensor_reduce(S[:, 320:384], CXt[:], axis=X, op=Op.max)

            # --- partition fold: 3 transposes + reduces ---
            ST = smallp.tile([P, 3, 128], i16)
            for t in range(3):
                nc.sync.dma_start(ST[:, t], S[:, 128 * t:128 * (t + 1)],
                                  transpose=True)
            # Y: rows (pl f j h); reduce over all 128 contributors
            FY = smallp.tile([P, 2], i16)
            nc.vector.tensor_reduce(FY[:, 0:1], ST[:, 0], axis=X, op=Op.min)
            nc.vector.tensor_reduce(FY[:, 1:2], ST[:, 1], axis=X, op=Op.max)
            # X: rows 0:64 = min (pl f j), 64:128 = max; contributors split by
            # parity g = q&1 (halfword h2 = g); output cols = g
            FX = smallp.tile([P, 2], i16)
            nc.vector.tensor_reduce(
                FX[0:64, :], ST[0:64, 2].rearrange("p (x g) -> p g x", g=2),
                axis=X, op=Op.min)
            nc.vector.tensor_reduce(
                FX[64:128, :], ST[64:128, 2].rearrange("p (x g) -> p g x", g=2),
                axis=X, op=Op.max)

            # --- fixups in f32 ---
            # mins: v==BIG (absent) -> 2147483648.0 ; maxes: v-1 == -1 -> -2^31
            BY = smallp.tile([P, 2], f32)
            BX = smallp.tile([P, 2], f32)
            fy = smallp.tile([P, 2], f32)
            fx = smallp.tile([P, 2], f32)
            nc.vector.tensor_copy(BY[:], FY[:])
            nc.vector.tensor_copy(BX[:], FX[:])
            nc.vector.tensor_scalar(BY[:, 1:2], BY[:, 1:2], 1, 0, Op.subtract, Op.add)
            nc.vector.tensor_scalar(BX[64:128, :], BX[64:128, :], 1, 0,
                                    Op.subtract, Op.add)
            nc.vector.tensor_scalar(fy[:, 0:1], BY[:, 0:1], 32767.0, 2147450880.0,
                                    Op.is_equal, Op.mult)
            nc.vector.tensor_scalar(fy[:, 1:2], BY[:, 1:2], -1.0, -2147483647.0,
                                    Op.is_equal, Op.mult)
            nc.vector.tensor_scalar(fx[0:64, :], BX[0:64, :], 32767.0, 2147450880.0,
                                    Op.is_equal, Op.mult)
            nc.vector.tensor_scalar(fx[64:128, :], BX[64:128, :], -1.0, -2147483647.0,
                                    Op.is_equal, Op.mult)
            nc.vector.tensor_tensor(BY[:], BY[:], fy[:], Op.add)
            nc.vector.tensor_tensor(BX[:], BX[:], fx[:], Op.add)

            # --- output DMAs ---
            # boxes[f, n, k]: k: 0 xmin, 1 ymin, 2 xmax, 3 ymax
            # Y rows p = h*64 + (pl*2+f)*16 + j ; n: pl=0: 31-16h-j, pl=1: 32+16h+j
            for col, k in [(0, 1), (1, 3)]:
                for h in range(2):
                    base = h * 64
                    nc.sync.dma_start(
                        dram_ap(boxes_out, k + 4 * (31 - 16 * h),
                                [(256, TL), (-4, 16)]),
                        BY[base:base + 32, col:col + 1])
                    nc.sync.dma_start(
                        dram_ap(boxes_out, k + 4 * (32 + 16 * h),
                                [(256, TL), (4, 16)]),
                        BY[base + 32:base + 64, col:col + 1])
            # X rows p = base + (pl*2+f)*16+j ; n: pl=0: 31-16g-j, pl=1: 32+16g+j
            for base, k in [(0, 0), (64, 2)]:
                for g in range(2):
                    nc.sync.dma_start(
                        dram_ap(boxes_out, k + 4 * (31 - 16 * g),
                                [(256, TL), (-4, 16)]),
                        BX[base:base + 32, g:g + 1])
                    nc.sync.dma_start(
                        dram_ap(boxes_out, k + 4 * (32 + 16 * g),
                                [(256, TL), (4, 16)]),
                        BX[base + 32:base + 64, g:g + 1])

    nc.finalize()
    if split_waits:
        _split_excess_waits(nc, mybir)
    return nc, tables


def _split_excess_waits(nc, mybir):
    """Hoist extra sem waits onto preceding NoOps.

    This walrus build rejects instructions carrying more sync-wait
    conditions than their ISA encoding holds (1 for TPB_CTRL ops and for
    Pool/core_v2 compute ops; 2 elsewhere, conservatively). Semantics are
    identical with the waits split onto dedicated NoOps just before the
    instruction.
    """
    n_split = 0
    for f in nc.m.functions:
        for bb in f.blocks:
            newl = []
            for ins in bb.instructions:
                si = ins.sync_info
                max_waits = 1
                if si and si.on_wait and len(si.on_wait) > max_waits:
                    waits = list(si.on_wait)
                    for j, w in enumerate(waits[max_waits:]):
                        nop = mybir.InstNoOp(
                            name=f"{ins.name}-w{j}", ins=[], outs=[],
                            engine=ins.engine,
                            sync_info=mybir.SyncInfo(on_wait=[w], on_update=[]))
                        newl.append(nop)
                        n_split += 1
                    ins.sync_info = mybir.SyncInfo(on_wait=waits[:max_waits],
                                                   on_update=si.on_update)
                newl.append(ins)
            bb.instructions = newl
    return n_split


def _get_program(TL, H, W, reps=1):
    key = (TL, H, W, reps)
    if key not in _BUILD_CACHE:
        _BUILD_CACHE[key] = _build_program(TL, H, W, reps=reps)
    return _BUILD_CACHE[key]


def kernel(segmentation, num_instances=None, **_ignored):
    from concourse.bass_utils import run_bass_kernel_spmd

    seg = np.asarray(segmentation)
    T, H, W = seg.shape
    assert T % _NCORES == 0
    TL = T // _NCORES
    nc, tables = _get_program(TL, H, W)

    seg = np.ascontiguousarray(seg, dtype=np.int32)
    in_maps = [{"seg": seg[i * TL:(i + 1) * TL], **tables}
               for i in range(_NCORES)]
    res = run_bass_kernel_spmd(nc, in_maps, list(range(_NCORES)))
    out = np.concatenate([res.results[i]["boxes"] for i in range(_NCORES)], axis=0)
    return out.astype(np.float32)


# revision 15
# speedup vs baseline: 1.3010x; 1.0051x over previous
"""Trainium2 Bass kernel for nn_BoxesFromMasks (per-frame segment bounding boxes).

Algorithm (per core, data-parallel over frames, TL=2 frames/core):
  Build per-pixel 64-bit one-hot bitmasks (2 u32 planes) of the instance id via
  the exponent-bit trick (ACT builds the f32 bit pattern of 2^k as an int, a
  second ACT converts value->u32, truncating out-of-range ids to 0):
    lo plane: id s in [0,32)  -> bit (31-s)
    hi plane: id s in [32,64) -> bit (s-32)
  Row masks:  OR-tree each 128-row chunk along columns (DVE), 16-wide leftovers
              folded once at extraction time.
  Col masks:  OR-accumulate chunks into acc[128,2,W]; pre-fold partitions
              128->64; DMA-transpose (u16); OR-tree the 64 contributors.
  Extraction (batched, no DRAM bounce): 16 u16 shift ops expand bits to
  E-tables, constant value-tables select coordinates via i16 mult/add, strided
  tensor_reduce min/max, one 3x128 transpose fold, and negative-stride output
  DMAs undo the bit-order permutation.
"""

import numpy as np

_T, _H, _W, _N = 16, 1024, 2048, 64
_NCORES = 8

_BUILD_CACHE = {}


def _build_program(TL, H, W, split_waits=True, reps=1):
    from contextlib import ExitStack

    import bass_rust
    import concourse.bass as bass
    import concourse.tile as tile
    import concourse.mybir as mybir
    from concourse.alu_op_type import AluOpType as Op

    f32 = mybir.dt.float32
    i32 = mybir.dt.int32
    u32 = mybir.dt.uint32
    u16 = mybir.dt.uint16
    i16 = mybir.dt.int16
    Copy = mybir.ActivationFunctionType.Copy
    X = mybir.AxisListType.X

    P = 128
    CH = H // P                   # row chunks per frame (8)
    KT = 4                        # transpose calls per frame (each 2048 u16 cols)
    MPER = 16                     # 128-col blocks per transpose call
    NSEG = 2                      # seg DMA splits per chunk
    LFT = 32                      # row-OR tree leftover width
    BIG = 32767
    assert TL == 2 and CH == 8 and W == 2048

    # ---- constant value tables (i16) ----
    pp = np.arange(P)
    # Y: value v(p, c) = 128c + p ; table shape [P, 64(pl f j), CH, 2h]
    yv = (128 * np.arange(CH)[None, :] + pp[:, None]).astype(np.int64)   # [P, CH]
    ty_mb = np.broadcast_to((yv - BIG)[:, None, :, None],
                            (P, 64, CH, 2)).astype(np.int16)
    ty_p1 = np.broadcast_to((yv + 1)[:, None, :, None],
                            (P, 64, CH, 2)).astype(np.int16)
    # X: value v(q, klo, m) = klo*1024 + 64m + (q>>1) ; table [P, 64(pl f j), 32(klo m)]
    klo = np.arange(2)
    mm = np.arange(MPER)
    xv = ((klo[:, None] * 1024 + 64 * mm[None, :]).reshape(-1)[None, :]
          + (pp[:, None] // 2)).astype(np.int64)                         # [P, 32]
    tx_mb = np.broadcast_to((xv - BIG)[:, None, :], (P, 64, 32)).astype(np.int16)
    tx_p1 = np.broadcast_to((xv + 1)[:, None, :], (P, 64, 32)).astype(np.int16)

    tables = {"ty_mb": ty_mb, "ty_p1": ty_p1, "tx_mb": tx_mb, "tx_p1": tx_p1}

    nc = bass.Bass()
    seg_in = nc.dram_tensor("seg", [TL, H, W], i32, kind="ExternalInput")
    boxes_out = nc.dram_tensor("boxes", [TL, 64, 4], f32, kind="ExternalOutput")
    d_tabs = {n: nc.dram_tensor(n, list(t.shape), i16, kind="ExternalInput")
              for n, t in tables.items()}

    def dram_ap(t, offset_elems, dims):
        """Manual DRAM AP: dims = [(stride_elems, count), ...]."""
        a2 = t[:].copy()
        a2.offset = offset_elems
        a2.ap = bass_rust.VecI64Pair([[s, n] for s, n in dims])
        return a2

    with tile.TileContext(nc) as tc, ExitStack() as ctx:
        constp = ctx.enter_context(tc.tile_pool(name="consts", bufs=1))
        segp = ctx.enter_context(tc.tile_pool(name="segp", bufs=3))
        ep = ctx.enter_context(tc.tile_pool(name="ep", bufs=2))
        accp = ctx.enter_context(tc.tile_pool(name="accp", bufs=2))
        accTp = ctx.enter_context(tc.tile_pool(name="accTp", bufs=2))
        rmp = ctx.enter_context(tc.tile_pool(name="rmp", bufs=1))
        trp = ctx.enter_context(tc.tile_pool(name="trp", bufs=2))
        xp = ctx.enter_context(tc.tile_pool(name="xp", bufs=1))
        smallp = ctx.enter_context(tc.tile_pool(name="smallp", bufs=1))

        c_ty_mb = constp.tile([P, 64, CH, 2], i16)
        c_ty_p1 = constp.tile([P, 64, CH, 2], i16)
        c_tx_mb = constp.tile([P, 64, 32], i16)
        c_tx_p1 = constp.tile([P, 64, 32], i16)
        for t, n in [(c_ty_mb, "ty_mb"), (c_ty_p1, "ty_p1"),
                     (c_tx_mb, "tx_mb"), (c_tx_p1, "tx_p1")]:
            nc.sync.dma_start(t[:], d_tabs[n][:])

        for _rep in range(reps):
            # rmask16: [p, pl, f, c, LFT] u32 (pl-major for contiguous planes)
            rmask16 = rmp.tile([P, 2, TL, CH, LFT], u32, tag="rmask16")
            # CMX: [q, pl, f, klo, m] u16 (compacted column masks)
            CMX = xp.tile([P, 2, TL, 2, MPER], u16, tag="cmx")

            # ================= main loop =================
            # seg tiles are created and their loads issued ahead of use so
            # next-frame loads precede this frame's transposes on the queue
            seg_tiles = {}

            def issue_seg(f, c):
                if f >= TL or (f, c) in seg_tiles:
                    return
                s = segp.tile([P, W], i32, tag="seg")
                rows = P // NSEG
                for k in range(NSEG):
                    nc.sync.dma_start(
                        s[rows * k:rows * (k + 1), :],
                        seg_in[f, c * P + rows * k:c * P + rows * (k + 1), :])
                seg_tiles[(f, c)] = s

            for f in range(TL):
                acc = accp.tile([P, 2, W], u32)
                prev_u = None
                for c in range(CH):
                    issue_seg(f, c)
                    s = seg_tiles.pop((f, c))

                    e = ep.tile([P, 2, W], i32)
                    # lo: bitpattern of 2^(31-s) = (158-s)<<23 ; hi: 2^(s-32) = (s+95)<<23
                    nc.scalar.activation(e[:, 0, :], s[:], Copy,
                                         bias=1325400064.0, scale=-8388608.0)
                    nc.gpsimd.tensor_scalar(e[:, 1, :], s[:], 8388608, 796917760,
                                            Op.mult, Op.add)
                    u = e[:].bitcast(u32)  # in-place cast target
                    nc.scalar.activation(u, e[:].bitcast(f32), Copy)

                    # column accumulate (DVE; only DVE has integer bitwise ops)
                    if c == 0:
                        prev_u = u
                    elif c == 1:
                        nc.vector.tensor_tensor(acc[:], u, prev_u, Op.bitwise_or)
                    else:
                        nc.vector.tensor_tensor(acc[:], u, acc[:], Op.bitwise_or)

                    # row masks: OR-tree along columns (DVE). In place, except
                    # chunk 0 whose u must stay intact for the c==1 accumulate.
                    if c == 0:
                        tr0 = trp.tile([P, 2, W // 2], u32, tag="tr0")
                        base = tr0[:]
                    else:
                        base = e[:, :, 0:W // 2].bitcast(u32)
                    w = W // 2
                    nc.vector.tensor_tensor(base[:, :, 0:w], u[:, :, 0:w],
                                            u[:, :, w:2 * w], Op.bitwise_or)
                    w //= 2
                    while w >= LFT:
                        nc.vector.tensor_tensor(base[:, :, 0:w], base[:, :, 0:w],
                                                base[:, :, w:2 * w], Op.bitwise_or)
                        w //= 2
                    nc.gpsimd.tensor_copy(rmask16[:, :, f, c, :], base[:, :, 0:LFT])

                # prefetch next frame's first chunks before the transposes so
                # their loads aren't queued behind acc-dependent triggers
                for c2 in range(3):
                    issue_seg(f + 1, c2)

                # ---- frame tail: transpose (u16), fold the 128 contributors
                accT = accTp.tile([P, KT, MPER, P], u16, tag="accT")
                a16 = acc[:].bitcast(u16).rearrange("p a b -> p (a b)")
                for k in range(KT):
                    nc.sync.dma_start(accT[:, k],
                                      a16[:, 2048 * k:2048 * (k + 1)],
                                      transpose=True)
                w = 64
                while w >= 1:
                    nc.vector.tensor_tensor(accT[:, :, :, 0:w],
                                            accT[:, :, :, 0:w],
                                            accT[:, :, :, w:2 * w],
                                            Op.bitwise_or)
                    w //= 2
                # compact: CMX[q, pl, f, klo, m] <- accT[q, (pl,klo), m, 0]
                nc.vector.tensor_copy(
                    CMX[:, :, f, :, :],
                    accT[:, :, :, 0].rearrange("q (pl klo) m -> q pl klo m",
                                               pl=2, klo=2))

            # ================= extraction =================
            # fold rowmask leftovers [..., LFT] -> [..., 1]
            rmf = rmask16[:].rearrange("p pl f c w -> p (pl f c) w")
            w = LFT // 2
            while w >= 1:
                nc.vector.tensor_tensor(rmf[:, :, 0:w], rmf[:, :, 0:w],
                                        rmf[:, :, w:2 * w], Op.bitwise_or)
                w //= 2

            # --- Y side ---
            # rm u16 view: [p, pl, f, c, h]  (h = u16 half; bit b32 = 16h + j)
            rmv = rmask16[:].bitcast(u16)[:, :, :, :, 0:2]
            rm_e = rmv.rearrange("p pl f c h -> p (pl f) c h")
            Ey = xp.tile([P, 2, TL, 16, CH, 2], i16, tag="ey")
            for j in range(16):
                nc.vector.tensor_scalar(
                    Ey[:, :, :, j].rearrange("p pl f c h -> p (pl f) c h").bitcast(u16),
                    rm_e, j, 1, Op.logical_shift_right, Op.bitwise_and)

            ey_flat = Ey[:].rearrange("p pl f j c h -> p (pl f j) c h")
            CY = xp.tile([P, 64, CH, 2], i16, tag="cy")
            S = smallp.tile([P, 384], i16)
            # Y block layout: col = t*128 + h*64 + (pl f j)  (h-major for output DMAs)
            Sy = S[:, 0:256].rearrange("p (t h a) -> p t h a", t=2, h=2, a=64)
            # ymin: min over c of E*(v-BIG)+BIG
            nc.vector.tensor_tensor(CY[:], ey_flat, c_ty_mb[:], Op.mult)
            nc.vector.tensor_scalar(CY[:], CY[:], BIG, None, Op.add)
            nc.vector.tensor_reduce(Sy[:, 0], CY[:].rearrange("p a c h -> p h a c"),
                                    axis=X, op=Op.min)
            # ymax(+1): max over c of E*(v+1)
            nc.vector.tensor_tensor(CY[:], ey_flat, c_ty_p1[:], Op.mult)
            nc.vector.tensor_reduce(Sy[:, 1], CY[:].rearrange("p a c h -> p h a c"),
                                    axis=X, op=Op.max)

            # --- X side ---
            cmx_flat = CMX[:].rearrange("q pl f klo m -> q (pl f) (klo m)")
            Ex = xp.tile([P, 4, 16, 32], i16, tag="ex")
            for j in range(16):
                nc.vector.tensor_scalar(Ex[:, :, j].bitcast(u16), cmx_flat,
                                        j, 1, Op.logical_shift_right, Op.bitwise_and)
            ex_flat = Ex[:].rearrange("q a j km -> q (a j) km")
            CXt = xp.tile([P, 64, 32], i16, tag="cx")
            nc.vector.tensor_tensor(CXt[:], ex_flat, c_tx_mb[:], Op.mult)
            nc.vector.tensor_scalar(CXt[:], CXt[:], BIG, None, Op.add)
            nc.vector.tensor_reduce(S[:, 256:320], CXt[:], axis=X, op=Op.min)
            nc.vector.tensor_tensor(CXt[:], ex_flat, c_tx_p1[:], Op.mult)
            nc.vector.tensor_reduce(S[:, 320:384], CXt[:], axis=X, op=Op.max)

            # --- partition fold: 3 transposes + reduces ---
            ST = smallp.tile([P, 3, 128], i16)
            for t in range(3):
                eng = nc.scalar if t % 2 else nc.sync
                eng.dma_start(ST[:, t], S[:, 128 * t:128 * (t + 1)],
                              transpose=True)
            # Y: rows (pl f j h); reduce over all 128 contributors
            FY = smallp.tile([P, 2], i16)
            nc.vector.tensor_reduce(FY[:, 0:1], ST[:, 0], axis=X, op=Op.min)
            nc.vector.tensor_reduce(FY[:, 1:2], ST[:, 1], axis=X, op=Op.max)
            # X: rows 0:64 = min (pl f j), 64:128 = max; contributors split by
            # parity g = q&1 (halfword h2 = g); output cols = g
            FX = smallp.tile([P, 2], i16)
            nc.vector.tensor_reduce(
                FX[0:64, :], ST[0:64, 2].rearrange("p (x g) -> p g x", g=2),
                axis=X, op=Op.min)
            nc.vector.tensor_reduce(
                FX[64:128, :], ST[64:128, 2].rearrange("p (x g) -> p g x", g=2),
                axis=X, op=Op.max)

            # --- fixups in f32 ---
            # mins: v==BIG (absent) -> 2147483648.0 ; maxes: v-1 == -1 -> -2^31
            BY = smallp.tile([P, 2], f32)
            BX = smallp.tile([P, 2], f32)
            fy = smallp.tile([P, 2], f32)
            fx = smallp.tile([P, 2], f32)
            nc.vector.tensor_copy(BY[:], FY[:])
            nc.vector.tensor_copy(BX[:], FX[:])
            nc.vector.tensor_scalar(BY[:, 1:2], BY[:, 1:2], 1, 0, Op.subtract, Op.add)
            nc.vector.tensor_scalar(BX[64:128, :], BX[64:128, :], 1, 0,
                                    Op.subtract, Op.add)
            nc.vector.tensor_scalar(fy[:, 0:1], BY[:, 0:1], 32767.0, 2147450880.0,
                                    Op.is_equal, Op.mult)
            nc.vector.tensor_scalar(fy[:, 1:2], BY[:, 1:2], -1.0, -2147483647.0,
                                    Op.is_equal, Op.mult)
            nc.vector.tensor_scalar(fx[0:64, :], BX[0:64, :], 32767.0, 2147450880.0,
                                    Op.is_equal, Op.mult)
            nc.vector.tensor_scalar(fx[64:128, :], BX[64:128, :], -1.0, -2147483647.0,
                                    Op.is_equal, Op.mult)
            nc.vector.tensor_tensor(BY[:], BY[:], fy[:], Op.add)
            nc.vector.tensor_tensor(BX[:], BX[:], fx[:], Op.add)

            # --- output DMAs ---
            # boxes[f, n, k]: k: 0 xmin, 1 ymin, 2 xmax, 3 ymax
            # Y rows p = h*64 + (pl*2+f)*16 + j ; n: pl=0: 31-16h-j, pl=1: 32+16h+j
            outn = [0]

            def out_dma(dst, src):
                eng = nc.scalar if outn[0] % 2 else nc.sync
                outn[0] += 1
                eng.dma_start(dst, src)

            for col, k in [(0, 1), (1, 3)]:
                for h in range(2):
                    base = h * 64
                    out_dma(dram_ap(boxes_out, k + 4 * (31 - 16 * h),
                                    [(256, TL), (-4, 16)]),
                            BY[base:base + 32, col:col + 1])
                    out_dma(dram_ap(boxes_out, k + 4 * (32 + 16 * h),
                                    [(256, TL), (4, 16)]),
                            BY[base + 32:base + 64, col:col + 1])
            # X rows p = base + (pl*2+f)*16+j ; n: pl=0: 31-16g-j, pl=1: 32+16g+j
            for base, k in [(0, 0), (64, 2)]:
                for g in range(2):
                    out_dma(dram_ap(boxes_out, k + 4 * (31 - 16 * g),
                                    [(256, TL), (-4, 16)]),
                            BX[base:base + 32, g:g + 1])
                    out_dma(dram_ap(boxes_out, k + 4 * (32 + 16 * g),
                                    [(256, TL), (4, 16)]),
                            BX[base + 32:base + 64, g:g + 1])

    nc.finalize()
    if split_waits:
        _split_excess_waits(nc, mybir)
    return nc, tables


def _split_excess_waits(nc, mybir):
    """Hoist extra sem waits onto preceding NoOps.

    This walrus build rejects instructions carrying more sync-wait
    conditions than their ISA encoding holds (1 for TPB_CTRL ops and for
    Pool/core_v2 compute ops; 2 elsewhere, conservatively). Semantics are
    identical with the waits split onto dedicated NoOps just before the
    instruction.
    """
    n_split = 0
    for f in nc.m.functions:
        for bb in f.blocks:
            newl = []
            for ins in bb.instructions:
                si = ins.sync_info
                max_waits = 1
                if si and si.on_wait and len(si.on_wait) > max_waits:
                    waits = list(si.on_wait)
                    for j, w in enumerate(waits[max_waits:]):
                        nop = mybir.InstNoOp(
                            name=f"{ins.name}-w{j}", ins=[], outs=[],
                            engine=ins.engine,
                            sync_info=mybir.SyncInfo(on_wait=[w], on_update=[]))
                        newl.append(nop)
                        n_split += 1
                    ins.sync_info = mybir.SyncInfo(on_wait=waits[:max_waits],
                                                   on_update=si.on_update)
                newl.append(ins)
            bb.instructions = newl
    return n_split


def _get_program(TL, H, W, reps=1):
    key = (TL, H, W, reps)
    if key not in _BUILD_CACHE:
        _BUILD_CACHE[key] = _build_program(TL, H, W, reps=reps)
    return _BUILD_CACHE[key]


def kernel(segmentation, num_instances=None, **_ignored):
    from concourse.bass_utils import run_bass_kernel_spmd

    seg = np.asarray(segmentation)
    T, H, W = seg.shape
    assert T % _NCORES == 0
    TL = T // _NCORES
    nc, tables = _get_program(TL, H, W)

    seg = np.ascontiguousarray(seg, dtype=np.int32)
    in_maps = [{"seg": seg[i * TL:(i + 1) * TL], **tables}
               for i in range(_NCORES)]
    res = run_bass_kernel_spmd(nc, in_maps, list(range(_NCORES)))
    out = np.concatenate([res.results[i]["boxes"] for i in range(_NCORES)], axis=0)
    return out.astype(np.float32)


# revision 18
# speedup vs baseline: 1.3443x; 1.0333x over previous
"""Trainium2 Bass kernel for nn_BoxesFromMasks (per-frame segment bounding boxes).

Algorithm (per core, data-parallel over frames, TL=2 frames/core):
  Build per-pixel 64-bit one-hot bitmasks (2 u32 planes) of the instance id via
  the exponent-bit trick (ACT builds the f32 bit pattern of 2^k as an int, a
  second ACT converts value->u32, truncating out-of-range ids to 0):
    lo plane: id s in [0,32)  -> bit (31-s)
    hi plane: id s in [32,64) -> bit (s-32)
  Row masks:  OR-tree each 128-row chunk along columns (DVE), 16-wide leftovers
              folded once at extraction time.
  Col masks:  OR-accumulate chunks into acc[128,2,W]; pre-fold partitions
              128->64; DMA-transpose (u16); OR-tree the 64 contributors.
  Extraction (batched, no DRAM bounce): 16 u16 shift ops expand bits to
  E-tables, constant value-tables select coordinates via i16 mult/add, strided
  tensor_reduce min/max, one 3x128 transpose fold, and negative-stride output
  DMAs undo the bit-order permutation.
"""

import numpy as np

_T, _H, _W, _N = 16, 1024, 2048, 64
_NCORES = 8

_BUILD_CACHE = {}


def _build_program(TL, H, W, split_waits=True, reps=1):
    from contextlib import ExitStack

    import bass_rust
    import concourse.bass as bass
    import concourse.tile as tile
    import concourse.mybir as mybir
    from concourse.alu_op_type import AluOpType as Op

    f32 = mybir.dt.float32
    i32 = mybir.dt.int32
    u32 = mybir.dt.uint32
    u16 = mybir.dt.uint16
    i16 = mybir.dt.int16
    Copy = mybir.ActivationFunctionType.Copy
    X = mybir.AxisListType.X

    P = 128
    CH = H // P                   # row chunks per frame (8)
    KT = 4                        # transpose calls per frame (each 2048 u16 cols)
    MPER = 16                     # 128-col blocks per transpose call
    NSEG = 2                      # seg DMA splits per chunk
    LFT = 32                      # row-OR tree leftover width
    BIG = 32767
    assert TL == 2 and CH == 8 and W == 2048

    # ---- constant value tables (i16) ----
    pp = np.arange(P)
    # Y: value v(p, c) = 128c + p ; table shape [P, 64(pl f j), CH, 2h]
    yv = (128 * np.arange(CH)[None, :] + pp[:, None]).astype(np.int64)   # [P, CH]
    ty_mb = np.broadcast_to((yv - BIG)[:, None, :, None],
                            (P, 64, CH, 2)).astype(np.int16)
    ty_p1 = np.broadcast_to((yv + 1)[:, None, :, None],
                            (P, 64, CH, 2)).astype(np.int16)
    # X: value v(q, klo, m) = klo*1024 + 64m + (q>>1) ; table [P, 64(pl f j), 32(klo m)]
    klo = np.arange(2)
    mm = np.arange(MPER)
    xv = ((klo[:, None] * 1024 + 64 * mm[None, :]).reshape(-1)[None, :]
          + (pp[:, None] // 2)).astype(np.int64)                         # [P, 32]
    tx_mb = np.broadcast_to((xv - BIG)[:, None, :], (P, 64, 32)).astype(np.int16)
    tx_p1 = np.broadcast_to((xv + 1)[:, None, :], (P, 64, 32)).astype(np.int16)

    tables = {"ty_mb": ty_mb, "ty_p1": ty_p1, "tx_mb": tx_mb, "tx_p1": tx_p1}

    nc = bass.Bass()
    seg_in = nc.dram_tensor("seg", [TL, H, W], i32, kind="ExternalInput")
    boxes_out = nc.dram_tensor("boxes", [TL, 64, 4], f32, kind="ExternalOutput")
    d_tabs = {n: nc.dram_tensor(n, list(t.shape), i16, kind="ExternalInput")
              for n, t in tables.items()}

    def dram_ap(t, offset_elems, dims):
        """Manual DRAM AP: dims = [(stride_elems, count), ...]."""
        a2 = t[:].copy()
        a2.offset = offset_elems
        a2.ap = bass_rust.VecI64Pair([[s, n] for s, n in dims])
        return a2

    with tile.TileContext(nc) as tc, ExitStack() as ctx:
        constp = ctx.enter_context(tc.tile_pool(name="consts", bufs=1))
        segp = ctx.enter_context(tc.tile_pool(name="segp", bufs=3))
        ep = ctx.enter_context(tc.tile_pool(name="ep", bufs=2))
        accp = ctx.enter_context(tc.tile_pool(name="accp", bufs=2))
        accTp = ctx.enter_context(tc.tile_pool(name="accTp", bufs=2))
        rmp = ctx.enter_context(tc.tile_pool(name="rmp", bufs=1))
        trp = ctx.enter_context(tc.tile_pool(name="trp", bufs=2))
        xp = ctx.enter_context(tc.tile_pool(name="xp", bufs=1))
        smallp = ctx.enter_context(tc.tile_pool(name="smallp", bufs=1))

        c_ty_mb = constp.tile([P, 64, CH, 2], i16)
        c_ty_p1 = constp.tile([P, 64, CH, 2], i16)
        c_tx_mb = constp.tile([P, 64, 32], i16)
        c_tx_p1 = constp.tile([P, 64, 32], i16)
        const_loaded = [False]

        def load_consts():
            if const_loaded[0]:
                return
            const_loaded[0] = True
            for t, n in [(c_ty_mb, "ty_mb"), (c_ty_p1, "ty_p1"),
                         (c_tx_mb, "tx_mb"), (c_tx_p1, "tx_p1")]:
                nc.scalar.dma_start(t[:], d_tabs[n][:])

        for _rep in range(reps):
            # rmask16: [p, pl, f, c, LFT] u32 (pl-major for contiguous planes)
            rmask16 = rmp.tile([P, 2, TL, CH, LFT], u32, tag="rmask16")
            # CMX: [q, pl, f, klo, m] u16 (compacted column masks)
            CMX = xp.tile([P, 2, TL, 2, MPER], u16, tag="cmx")

            # ================= main loop =================
            # seg tiles are created and their loads issued ahead of use so
            # next-frame loads precede this frame's transposes on the queue
            seg_tiles = {}

            def issue_seg(f, c):
                if f >= TL or (f, c) in seg_tiles:
                    return
                s = segp.tile([P, W], i32, tag="seg")
                rows = P // NSEG
                for k in range(NSEG):
                    nc.sync.dma_start(
                        s[rows * k:rows * (k + 1), :],
                        seg_in[f, c * P + rows * k:c * P + rows * (k + 1), :])
                seg_tiles[(f, c)] = s

            for c2 in range(3):
                issue_seg(0, c2)
            load_consts()

            for f in range(TL):
                acc = accp.tile([P, 2, W], u32)
                prev_u = None
                for c in range(CH):
                    issue_seg(f, c)
                    s = seg_tiles.pop((f, c))

                    e = ep.tile([P, 2, W], i32)
                    # lo: bitpattern of 2^(31-s) = (158-s)<<23 ; hi: 2^(s-32) = (s+95)<<23
                    nc.scalar.activation(e[:, 0, :], s[:], Copy,
                                         bias=1325400064.0, scale=-8388608.0)
                    nc.gpsimd.tensor_scalar(e[:, 1, :], s[:], 8388608, 796917760,
                                            Op.mult, Op.add)
                    u = e[:].bitcast(u32)  # in-place cast target
                    nc.scalar.activation(u, e[:].bitcast(f32), Copy)

                    # column accumulate (DVE; only DVE has integer bitwise ops)
                    if c == 0:
                        prev_u = u
                    elif c == 1:
                        nc.vector.tensor_tensor(acc[:], u, prev_u, Op.bitwise_or)
                    else:
                        nc.vector.tensor_tensor(acc[:], u, acc[:], Op.bitwise_or)

                    # row masks: OR-tree along columns (DVE). In place, except
                    # chunk 0 whose u must stay intact for the c==1 accumulate.
                    if c == 0:
                        tr0 = trp.tile([P, 2, W // 2], u32, tag="tr0")
                        base = tr0[:]
                    else:
                        base = e[:, :, 0:W // 2].bitcast(u32)
                    w = W // 2
                    nc.vector.tensor_tensor(base[:, :, 0:w], u[:, :, 0:w],
                                            u[:, :, w:2 * w], Op.bitwise_or)
                    w //= 2
                    while w > LFT:
                        nc.vector.tensor_tensor(base[:, :, 0:w], base[:, :, 0:w],
                                                base[:, :, w:2 * w], Op.bitwise_or)
                        w //= 2
                    # last level writes the leftovers straight into rmask16
                    nc.vector.tensor_tensor(rmask16[:, :, f, c, :],
                                            base[:, :, 0:LFT],
                                            base[:, :, LFT:2 * LFT], Op.bitwise_or)

                # prefetch next frame's first chunks before the transposes so
                # their loads aren't queued behind acc-dependent triggers
                for c2 in range(3):
                    issue_seg(f + 1, c2)

                # ---- frame tail: transpose (u16), fold the 128 contributors
                accT = accTp.tile([P, KT, MPER, P], u16, tag="accT")
                a16 = acc[:].bitcast(u16).rearrange("p a b -> p (a b)")
                for k in range(KT):
                    nc.sync.dma_start(accT[:, k],
                                      a16[:, 2048 * k:2048 * (k + 1)],
                                      transpose=True)
                w = 64
                while w >= 1:
                    nc.vector.tensor_tensor(accT[:, :, :, 0:w],
                                            accT[:, :, :, 0:w],
                                            accT[:, :, :, w:2 * w],
                                            Op.bitwise_or)
                    w //= 2
                # compact: CMX[q, pl, f, klo, m] <- accT[q, (pl,klo), m, 0]
                nc.vector.tensor_copy(
                    CMX[:, :, f, :, :],
                    accT[:, :, :, 0].rearrange("q (pl klo) m -> q pl klo m",
                                               pl=2, klo=2))

            # ================= extraction =================
            # fold rowmask leftovers [..., LFT] -> [..., 1]
            rmf = rmask16[:].rearrange("p pl f c w -> p (pl f c) w")
            w = LFT // 2
            while w >= 1:
                nc.vector.tensor_tensor(rmf[:, :, 0:w], rmf[:, :, 0:w],
                                        rmf[:, :, w:2 * w], Op.bitwise_or)
                w //= 2

            # --- Y side ---
            # rm u16 view: [p, pl, f, c, h]  (h = u16 half; bit b32 = 16h + j)
            rmv = rmask16[:].bitcast(u16)[:, :, :, :, 0:2]
            rm_e = rmv.rearrange("p pl f c h -> p (pl f) c h")
            Ey = xp.tile([P, 2, TL, 16, CH, 2], i16, tag="ey")
            for j in range(16):
                nc.vector.tensor_scalar(
                    Ey[:, :, :, j].rearrange("p pl f c h -> p (pl f) c h").bitcast(u16),
                    rm_e, j, 1, Op.logical_shift_right, Op.bitwise_and)

            ey_flat = Ey[:].rearrange("p pl f j c h -> p (pl f j) c h")
            CY = xp.tile([P, 64, CH, 2], i16, tag="cy")
            S = smallp.tile([P, 384], i16)
            # Y block layout: col = t*128 + h*64 + (pl f j)  (h-major for output DMAs)
            Sy = S[:, 0:256].rearrange("p (t h a) -> p t h a", t=2, h=2, a=64)
            # ymin: min over c of E*(v-BIG)+BIG
            nc.vector.tensor_tensor(CY[:], ey_flat, c_ty_mb[:], Op.mult)
            nc.vector.tensor_scalar(CY[:], CY[:], BIG, None, Op.add)
            nc.vector.tensor_reduce(Sy[:, 0], CY[:].rearrange("p a c h -> p h a c"),
                                    axis=X, op=Op.min)
            # ymax(+1): max over c of E*(v+1)
            nc.vector.tensor_tensor(CY[:], ey_flat, c_ty_p1[:], Op.mult)
            nc.vector.tensor_reduce(Sy[:, 1], CY[:].rearrange("p a c h -> p h a c"),
                                    axis=X, op=Op.max)

            # --- X side ---
            cmx_flat = CMX[:].rearrange("q pl f klo m -> q (pl f) (klo m)")
            Ex = xp.tile([P, 4, 16, 32], i16, tag="ex")
            for j in range(16):
                nc.vector.tensor_scalar(Ex[:, :, j].bitcast(u16), cmx_flat,
                                        j, 1, Op.logical_shift_right, Op.bitwise_and)
            ex_flat = Ex[:].rearrange("q a j km -> q (a j) km")
            CXt = xp.tile([P, 64, 32], i16, tag="cx")
            nc.vector.tensor_tensor(CXt[:], ex_flat, c_tx_mb[:], Op.mult)
            nc.vector.tensor_scalar(CXt[:], CXt[:], BIG, None, Op.add)
            nc.vector.tensor_reduce(S[:, 256:320], CXt[:], axis=X, op=Op.min)
            nc.vector.tensor_tensor(CXt[:], ex_flat, c_tx_p1[:], Op.mult)
            nc.vector.tensor_reduce(S[:, 320:384], CXt[:], axis=X, op=Op.max)

            # --- partition fold: 3 transposes + reduces ---
            ST = smallp.tile([P, 3, 128], i16)
            for t in range(3):
                eng = nc.scalar if t % 2 else nc.sync
                eng.dma_start(ST[:, t], S[:, 128 * t:128 * (t + 1)],
                              transpose=True)
            # Y: rows (pl f j h); reduce over all 128 contributors
            FY = smallp.tile([P, 2], i16)
            nc.vector.tensor_reduce(FY[:, 0:1], ST[:, 0], axis=X, op=Op.min)
            nc.vector.tensor_reduce(FY[:, 1:2], ST[:, 1], axis=X, op=Op.max)
            # X: rows 0:64 = min (pl f j), 64:128 = max; contributors split by
            # parity g = q&1 (halfword h2 = g); output cols = g
            FX = smallp.tile([P, 2], i16)
            nc.vector.tensor_reduce(
                FX[0:64, :], ST[0:64, 2].rearrange("p (x g) -> p g x", g=2),
                axis=X, op=Op.min)
            nc.vector.tensor_reduce(
                FX[64:128, :], ST[64:128, 2].rearrange("p (x g) -> p g x", g=2),
                axis=X, op=Op.max)

            # --- fixups in f32 ---
            # mins: v==BIG (absent) -> 2147483648.0 ; maxes: v-1 == -1 -> -2^31
            BY = smallp.tile([P, 2], f32)
            BX = smallp.tile([P, 2], f32)
            fy = smallp.tile([P, 2], f32)
            fx = smallp.tile([P, 2], f32)
            nc.vector.tensor_copy(BY[:], FY[:])
            nc.vector.tensor_copy(BX[:], FX[:])
            nc.vector.tensor_scalar(BY[:, 1:2], BY[:, 1:2], 1, 0, Op.subtract, Op.add)
            nc.vector.tensor_scalar(BX[64:128, :], BX[64:128, :], 1, 0,
                                    Op.subtract, Op.add)
            nc.vector.tensor_scalar(fy[:, 0:1], BY[:, 0:1], 32767.0, 2147450880.0,
                                    Op.is_equal, Op.mult)
            nc.vector.tensor_scalar(fy[:, 1:2], BY[:, 1:2], -1.0, -2147483647.0,
                                    Op.is_equal, Op.mult)
            nc.vector.tensor_scalar(fx[0:64, :], BX[0:64, :], 32767.0, 2147450880.0,
                                    Op.is_equal, Op.mult)
            nc.vector.tensor_scalar(fx[64:128, :], BX[64:128, :], -1.0, -2147483647.0,
                                    Op.is_equal, Op.mult)
            nc.vector.tensor_tensor(BY[:], BY[:], fy[:], Op.add)
            nc.vector.tensor_tensor(BX[:], BX[:], fx[:], Op.add)

            # --- output DMAs ---
            # boxes[f, n, k]: k: 0 xmin, 1 ymin, 2 xmax, 3 ymax
            # Y rows p = h*64 + (pl*2+f)*16 + j ; n: pl=0: 31-16h-j, pl=1: 32+16h+j
            outn = [0]

            def out_dma(dst, src):
                eng = nc.scalar if outn[0] % 2 else nc.sync
                outn[0] += 1
                eng.dma_start(dst, src)

            for col, k in [(0, 1), (1, 3)]:
                for h in range(2):
                    base = h * 64
                    out_dma(dram_ap(boxes_out, k + 4 * (31 - 16 * h),
                                    [(256, TL), (-4, 16)]),
                            BY[base:base + 32, col:col + 1])
                    out_dma(dram_ap(boxes_out, k + 4 * (32 + 16 * h),
                                    [(256, TL), (4, 16)]),
                            BY[base + 32:base + 64, col:col + 1])
            # X rows p = base + (pl*2+f)*16+j ; n: pl=0: 31-16g-j, pl=1: 32+16g+j
            for base, k in [(0, 0), (64, 2)]:
                for g in range(2):
                    out_dma(dram_ap(boxes_out, k + 4 * (31 - 16 * g),
                                    [(256, TL), (-4, 16)]),
                            BX[base:base + 32, g:g + 1])
                    out_dma(dram_ap(boxes_out, k + 4 * (32 + 16 * g),
                                    [(256, TL), (4, 16)]),
                            BX[base + 32:base + 64, g:g + 1])

    nc.finalize()
    if split_waits:
        _split_excess_waits(nc, mybir)
    return nc, tables


def _split_excess_waits(nc, mybir):
    """Hoist extra sem waits onto preceding NoOps.

    This walrus build rejects instructions carrying more sync-wait
    conditions than their ISA encoding holds (1 for TPB_CTRL ops and for
    Pool/core_v2 compute ops; 2 elsewhere, conservatively). Semantics are
    identical with the waits split onto dedicated NoOps just before the
    instruction.
    """
    n_split = 0
    for f in nc.m.functions:
        for bb in f.blocks:
            newl = []
            for ins in bb.instructions:
                si = ins.sync_info
                max_waits = 1
                if si and si.on_wait and len(si.on_wait) > max_waits:
                    waits = list(si.on_wait)
                    for j, w in enumerate(waits[max_waits:]):
                        nop = mybir.InstNoOp(
                            name=f"{ins.name}-w{j}", ins=[], outs=[],
                            engine=ins.engine,
                            sync_info=mybir.SyncInfo(on_wait=[w], on_update=[]))
                        newl.append(nop)
                        n_split += 1
                    ins.sync_info = mybir.SyncInfo(on_wait=waits[:max_waits],
                                                   on_update=si.on_update)
                newl.append(ins)
            bb.instructions = newl
    return n_split


def _get_program(TL, H, W, reps=1):
    key = (TL, H, W, reps)
    if key not in _BUILD_CACHE:
        _BUILD_CACHE[key] = _build_program(TL, H, W, reps=reps)
    return _BUILD_CACHE[key]


def kernel(segmentation, num_instances=None, **_ignored):
    from concourse.bass_utils import run_bass_kernel_spmd

    seg = np.asarray(segmentation)
    T, H, W = seg.shape
    assert T % _NCORES == 0
    TL = T // _NCORES
    nc, tables = _get_program(TL, H, W)

    seg = np.ascontiguousarray(seg, dtype=np.int32)
    in_maps = [{"seg": seg[i * TL:(i + 1) * TL], **tables}
               for i in range(_NCORES)]
    res = run_bass_kernel_spmd(nc, in_maps, list(range(_NCORES)))
    out = np.concatenate([res.results[i]["boxes"] for i in range(_NCORES)], axis=0)
    return out.astype(np.float32)


# revision 19
# speedup vs baseline: 1.4642x; 1.0892x over previous
"""Trainium2 Bass kernel for nn_BoxesFromMasks (per-frame segment bounding boxes).

Algorithm (per core, data-parallel over frames, TL=2 frames/core):
  Build per-pixel 64-bit one-hot bitmasks (2 u32 planes) of the instance id via
  the exponent-bit trick (ACT builds the f32 bit pattern of 2^k as an int, a
  second ACT converts value->u32, truncating out-of-range ids to 0):
    lo plane: id s in [0,32)  -> bit (31-s)
    hi plane: id s in [32,64) -> bit (s-32)
  Row masks:  OR-tree each 128-row chunk along columns (DVE), 16-wide leftovers
              folded once at extraction time.
  Col masks:  OR-accumulate chunks into acc[128,2,W]; pre-fold partitions
              128->64; DMA-transpose (u16); OR-tree the 64 contributors.
  Extraction (batched, no DRAM bounce): 16 u16 shift ops expand bits to
  E-tables, constant value-tables select coordinates via i16 mult/add, strided
  tensor_reduce min/max, one 3x128 transpose fold, and negative-stride output
  DMAs undo the bit-order permutation.
"""

import numpy as np

_T, _H, _W, _N = 16, 1024, 2048, 64
_NCORES = 8

_BUILD_CACHE = {}


def _build_program(TL, H, W, split_waits=True, reps=1):
    from contextlib import ExitStack

    import bass_rust
    import concourse.bass as bass
    import concourse.tile as tile
    import concourse.mybir as mybir
    from concourse.alu_op_type import AluOpType as Op

    f32 = mybir.dt.float32
    i32 = mybir.dt.int32
    u32 = mybir.dt.uint32
    u16 = mybir.dt.uint16
    i16 = mybir.dt.int16
    Copy = mybir.ActivationFunctionType.Copy
    X = mybir.AxisListType.X

    P = 128
    CH = H // P                   # row chunks per frame (8)
    KT = 4                        # transpose calls per frame (each 2048 u16 cols)
    MPER = 16                     # 128-col blocks per transpose call
    NSEG = 2                      # seg DMA splits per chunk
    LFT = 32                      # row-OR tree leftover width
    BIG = 32767
    assert TL == 2 and CH == 8 and W == 2048

    # ---- constant value tables (i16) ----
    pp = np.arange(P)
    # Y: value v(p, c) = 128c + p ; table shape [P, 64(pl f j), CH, 2h]
    yv = (128 * np.arange(CH)[None, :] + pp[:, None]).astype(np.int64)   # [P, CH]
    ty_mb = np.broadcast_to((yv - BIG)[:, None, :, None],
                            (P, 64, CH, 2)).astype(np.int16)
    ty_p1 = np.broadcast_to((yv + 1)[:, None, :, None],
                            (P, 64, CH, 2)).astype(np.int16)
    # X: value v(q, klo, m) = klo*1024 + 64m + (q>>1) ; table [P, 64(pl f j), 32(klo m)]
    klo = np.arange(2)
    mm = np.arange(MPER)
    xv = ((klo[:, None] * 1024 + 64 * mm[None, :]).reshape(-1)[None, :]
          + (pp[:, None] // 2)).astype(np.int64)                         # [P, 32]
    tx_mb = np.broadcast_to((xv - BIG)[:, None, :], (P, 64, 32)).astype(np.int16)
    tx_p1 = np.broadcast_to((xv + 1)[:, None, :], (P, 64, 32)).astype(np.int16)

    tables = {"ty_mb": ty_mb, "ty_p1": ty_p1, "tx_mb": tx_mb, "tx_p1": tx_p1}

    nc = bass.Bass()
    seg_in = nc.dram_tensor("seg", [TL, H, W], i32, kind="ExternalInput")
    boxes_out = nc.dram_tensor("boxes", [TL, 64, 4], f32, kind="ExternalOutput")
    d_tabs = {n: nc.dram_tensor(n, list(t.shape), i16, kind="ExternalInput")
              for n, t in tables.items()}

    def dram_ap(t, offset_elems, dims):
        """Manual DRAM AP: dims = [(stride_elems, count), ...]."""
        a2 = t[:].copy()
        a2.offset = offset_elems
        a2.ap = bass_rust.VecI64Pair([[s, n] for s, n in dims])
        return a2

    with tile.TileContext(nc) as tc, ExitStack() as ctx:
        constp = ctx.enter_context(tc.tile_pool(name="consts", bufs=1))
        segp = ctx.enter_context(tc.tile_pool(name="segp", bufs=3))
        ep = ctx.enter_context(tc.tile_pool(name="ep", bufs=3))
        accp = ctx.enter_context(tc.tile_pool(name="accp", bufs=2))
        accTp = ctx.enter_context(tc.tile_pool(name="accTp", bufs=2))
        rmp = ctx.enter_context(tc.tile_pool(name="rmp", bufs=1))
        trp = ctx.enter_context(tc.tile_pool(name="trp", bufs=2))
        xp = ctx.enter_context(tc.tile_pool(name="xp", bufs=1))
        smallp = ctx.enter_context(tc.tile_pool(name="smallp", bufs=1))

        c_ty_mb = constp.tile([P, 64, CH, 2], i16)
        c_ty_p1 = constp.tile([P, 64, CH, 2], i16)
        c_tx_mb = constp.tile([P, 64, 32], i16)
        c_tx_p1 = constp.tile([P, 64, 32], i16)
        const_loaded = [False]

        def load_consts():
            if const_loaded[0]:
                return
            const_loaded[0] = True
            for t, n in [(c_ty_mb, "ty_mb"), (c_ty_p1, "ty_p1"),
                         (c_tx_mb, "tx_mb"), (c_tx_p1, "tx_p1")]:
                nc.scalar.dma_start(t[:], d_tabs[n][:])

        for _rep in range(reps):
            # rmask16: [p, pl, f, c, LFT] u32 (pl-major for contiguous planes)
            rmask16 = rmp.tile([P, 2, TL, CH, LFT], u32, tag="rmask16")
            # CMX: [q, pl, f, klo, m] u16 (compacted column masks)
            CMX = xp.tile([P, 2, TL, 2, MPER], u16, tag="cmx")

            # ================= main loop =================
            # seg tiles are created and their loads issued ahead of use so
            # next-frame loads precede this frame's transposes on the queue
            seg_tiles = {}

            def issue_seg(f, c):
                if f >= TL or (f, c) in seg_tiles:
                    return
                s = segp.tile([P, W], i32, tag="seg")
                rows = P // NSEG
                for k in range(NSEG):
                    nc.sync.dma_start(
                        s[rows * k:rows * (k + 1), :],
                        seg_in[f, c * P + rows * k:c * P + rows * (k + 1), :])
                seg_tiles[(f, c)] = s

            for c2 in range(3):
                issue_seg(0, c2)
            load_consts()

            for f in range(TL):
                acc = accp.tile([P, 2, W], u32)
                prev_u = None
                for c in range(CH):
                    issue_seg(f, c)
                    s = seg_tiles.pop((f, c))

                    e = ep.tile([P, 2, W], i32)
                    # lo: bitpattern of 2^(31-s) = (158-s)<<23 ; hi: 2^(s-32) = (s+95)<<23
                    nc.scalar.activation(e[:, 0, :], s[:], Copy,
                                         bias=1325400064.0, scale=-8388608.0)
                    nc.gpsimd.tensor_scalar(e[:, 1, :], s[:], 8388608, 796917760,
                                            Op.mult, Op.add)
                    u = e[:].bitcast(u32)  # in-place cast target
                    nc.scalar.activation(u, e[:].bitcast(f32), Copy)

                    # column accumulate (DVE; only DVE has integer bitwise ops)
                    if c == 0:
                        prev_u = u
                    elif c == 1:
                        nc.vector.tensor_tensor(acc[:], u, prev_u, Op.bitwise_or)
                    else:
                        nc.vector.tensor_tensor(acc[:], u, acc[:], Op.bitwise_or)

                    # row masks: OR-tree along columns (DVE). In place, except
                    # chunk 0 whose u must stay intact for the c==1 accumulate.
                    if c == 0:
                        tr0 = trp.tile([P, 2, W // 2], u32, tag="tr0")
                        base = tr0[:]
                    else:
                        base = e[:, :, 0:W // 2].bitcast(u32)
                    w = W // 2
                    nc.vector.tensor_tensor(base[:, :, 0:w], u[:, :, 0:w],
                                            u[:, :, w:2 * w], Op.bitwise_or)
                    w //= 2
                    while w > LFT:
                        nc.vector.tensor_tensor(base[:, :, 0:w], base[:, :, 0:w],
                                                base[:, :, w:2 * w], Op.bitwise_or)
                        w //= 2
                    # last level writes the leftovers straight into rmask16
                    nc.vector.tensor_tensor(rmask16[:, :, f, c, :],
                                            base[:, :, 0:LFT],
                                            base[:, :, LFT:2 * LFT], Op.bitwise_or)

                # prefetch next frame's first chunks before the transposes so
                # their loads aren't queued behind acc-dependent triggers
                for c2 in range(3):
                    issue_seg(f + 1, c2)

                # ---- frame tail: transpose (u16), fold the 128 contributors
                accT = accTp.tile([P, KT, MPER, P], u16, tag="accT")
                a16 = acc[:].bitcast(u16).rearrange("p a b -> p (a b)")
                for k in range(KT):
                    nc.sync.dma_start(accT[:, k],
                                      a16[:, 2048 * k:2048 * (k + 1)],
                                      transpose=True)
                w = 64
                while w >= 1:
                    nc.vector.tensor_tensor(accT[:, :, :, 0:w],
                                            accT[:, :, :, 0:w],
                                            accT[:, :, :, w:2 * w],
                                            Op.bitwise_or)
                    w //= 2
                # compact: CMX[q, pl, f, klo, m] <- accT[q, (pl,klo), m, 0]
                nc.vector.tensor_copy(
                    CMX[:, :, f, :, :],
                    accT[:, :, :, 0].rearrange("q (pl klo) m -> q pl klo m",
                                               pl=2, klo=2))

            # ================= extraction =================
            # fold rowmask leftovers [..., LFT] -> [..., 1]
            rmf = rmask16[:].rearrange("p pl f c w -> p (pl f c) w")
            w = LFT // 2
            while w >= 1:
                nc.vector.tensor_tensor(rmf[:, :, 0:w], rmf[:, :, 0:w],
                                        rmf[:, :, w:2 * w], Op.bitwise_or)
                w //= 2

            # --- Y side ---
            # rm u16 view: [p, pl, f, c, h]  (h = u16 half; bit b32 = 16h + j)
            rmv = rmask16[:].bitcast(u16)[:, :, :, :, 0:2]
            rm_e = rmv.rearrange("p pl f c h -> p (pl f) c h")
            Ey = xp.tile([P, 2, TL, 16, CH, 2], i16, tag="ey")
            for j in range(16):
                nc.vector.tensor_scalar(
                    Ey[:, :, :, j].rearrange("p pl f c h -> p (pl f) c h").bitcast(u16),
                    rm_e, j, 1, Op.logical_shift_right, Op.bitwise_and)

            ey_flat = Ey[:].rearrange("p pl f j c h -> p (pl f j) c h")
            CY = xp.tile([P, 64, CH, 2], i16, tag="cy")
            S = smallp.tile([P, 384], i16)
            # Y block layout: col = t*128 + h*64 + (pl f j)  (h-major for output DMAs)
            Sy = S[:, 0:256].rearrange("p (t h a) -> p t h a", t=2, h=2, a=64)
            # ymin: min over c of E*(v-BIG)+BIG
            nc.vector.tensor_tensor(CY[:], ey_flat, c_ty_mb[:], Op.mult)
            nc.vector.tensor_scalar(CY[:], CY[:], BIG, None, Op.add)
            nc.vector.tensor_reduce(Sy[:, 0], CY[:].rearrange("p a c h -> p h a c"),
                                    axis=X, op=Op.min)
            # ymax(+1): max over c of E*(v+1)
            nc.vector.tensor_tensor(CY[:], ey_flat, c_ty_p1[:], Op.mult)
            nc.vector.tensor_reduce(Sy[:, 1], CY[:].rearrange("p a c h -> p h a c"),
                                    axis=X, op=Op.max)

            # --- X side ---
            cmx_flat = CMX[:].rearrange("q pl f klo m -> q (pl f) (klo m)")
            Ex = xp.tile([P, 4, 16, 32], i16, tag="ex")
            for j in range(16):
                nc.vector.tensor_scalar(Ex[:, :, j].bitcast(u16), cmx_flat,
                                        j, 1, Op.logical_shift_right, Op.bitwise_and)
            ex_flat = Ex[:].rearrange("q a j km -> q (a j) km")
            CXt = xp.tile([P, 64, 32], i16, tag="cx")
            nc.vector.tensor_tensor(CXt[:], ex_flat, c_tx_mb[:], Op.mult)
            nc.vector.tensor_scalar(CXt[:], CXt[:], BIG, None, Op.add)
            nc.vector.tensor_reduce(S[:, 256:320], CXt[:], axis=X, op=Op.min)
            nc.vector.tensor_tensor(CXt[:], ex_flat, c_tx_p1[:], Op.mult)
            nc.vector.tensor_reduce(S[:, 320:384], CXt[:], axis=X, op=Op.max)

            # --- partition fold: 3 transposes + reduces ---
            ST = smallp.tile([P, 3, 128], i16)
            for t in range(3):
                eng = nc.scalar if t % 2 else nc.sync
                eng.dma_start(ST[:, t], S[:, 128 * t:128 * (t + 1)],
                              transpose=True)
            # Y: rows (pl f j h); reduce over all 128 contributors
            FY = smallp.tile([P, 2], i16)
            nc.vector.tensor_reduce(FY[:, 0:1], ST[:, 0], axis=X, op=Op.min)
            nc.vector.tensor_reduce(FY[:, 1:2], ST[:, 1], axis=X, op=Op.max)
            # X: rows 0:64 = min (pl f j), 64:128 = max; contributors split by
            # parity g = q&1 (halfword h2 = g); output cols = g
            FX = smallp.tile([P, 2], i16)
            nc.vector.tensor_reduce(
                FX[0:64, :], ST[0:64, 2].rearrange("p (x g) -> p g x", g=2),
                axis=X, op=Op.min)
            nc.vector.tensor_reduce(
                FX[64:128, :], ST[64:128, 2].rearrange("p (x g) -> p g x", g=2),
                axis=X, op=Op.max)

            # --- fixups in f32 ---
            # mins: v==BIG (absent) -> 2147483648.0 ; maxes: v-1 == -1 -> -2^31
            BY = smallp.tile([P, 2], f32)
            BX = smallp.tile([P, 2], f32)
            fy = smallp.tile([P, 2], f32)
            fx = smallp.tile([P, 2], f32)
            nc.vector.tensor_copy(BY[:], FY[:])
            nc.vector.tensor_copy(BX[:], FX[:])
            nc.vector.tensor_scalar(BY[:, 1:2], BY[:, 1:2], 1, 0, Op.subtract, Op.add)
            nc.vector.tensor_scalar(BX[64:128, :], BX[64:128, :], 1, 0,
                                    Op.subtract, Op.add)
            nc.vector.tensor_scalar(fy[:, 0:1], BY[:, 0:1], 32767.0, 2147450880.0,
                                    Op.is_equal, Op.mult)
            nc.vector.tensor_scalar(fy[:, 1:2], BY[:, 1:2], -1.0, -2147483647.0,
                                    Op.is_equal, Op.mult)
            nc.vector.tensor_scalar(fx[0:64, :], BX[0:64, :], 32767.0, 2147450880.0,
                                    Op.is_equal, Op.mult)
            nc.vector.tensor_scalar(fx[64:128, :], BX[64:128, :], -1.0, -2147483647.0,
                                    Op.is_equal, Op.mult)
            nc.vector.tensor_tensor(BY[:], BY[:], fy[:], Op.add)
            nc.vector.tensor_tensor(BX[:], BX[:], fx[:], Op.add)

            # --- output DMAs ---
            # boxes[f, n, k]: k: 0 xmin, 1 ymin, 2 xmax, 3 ymax
            # Y rows p = h*64 + (pl*2+f)*16 + j ; n: pl=0: 31-16h-j, pl=1: 32+16h+j
            outn = [0]

            def out_dma(dst, src):
                eng = nc.scalar if outn[0] % 2 else nc.sync
                outn[0] += 1
                eng.dma_start(dst, src)

            for col, k in [(0, 1), (1, 3)]:
                for h in range(2):
                    base = h * 64
                    out_dma(dram_ap(boxes_out, k + 4 * (31 - 16 * h),
                                    [(256, TL), (-4, 16)]),
                            BY[base:base + 32, col:col + 1])
                    out_dma(dram_ap(boxes_out, k + 4 * (32 + 16 * h),
                                    [(256, TL), (4, 16)]),
                            BY[base + 32:base + 64, col:col + 1])
            # X rows p = base + (pl*2+f)*16+j ; n: pl=0: 31-16g-j, pl=1: 32+16g+j
            for base, k in [(0, 0), (64, 2)]:
                for g in range(2):
                    out_dma(dram_ap(boxes_out, k + 4 * (31 - 16 * g),
                                    [(256, TL), (-4, 16)]),
                            BX[base:base + 32, g:g + 1])
                    out_dma(dram_ap(boxes_out, k + 4 * (32 + 16 * g),
                                    [(256, TL), (4, 16)]),
                            BX[base + 32:base + 64, g:g + 1])

    nc.finalize()
    if split_waits:
        _split_excess_waits(nc, mybir)
    return nc, tables


def _split_excess_waits(nc, mybir):
    """Hoist extra sem waits onto preceding NoOps.

    This walrus build rejects instructions carrying more sync-wait
    conditions than their ISA encoding holds (1 for TPB_CTRL ops and for
    Pool/core_v2 compute ops; 2 elsewhere, conservatively). Semantics are
    identical with the waits split onto dedicated NoOps just before the
    instruction.
    """
    n_split = 0
    for f in nc.m.functions:
        for bb in f.blocks:
            newl = []
            for ins in bb.instructions:
                si = ins.sync_info
                max_waits = 1
                if si and si.on_wait and len(si.on_wait) > max_waits:
                    waits = list(si.on_wait)
                    for j, w in enumerate(waits[max_waits:]):
                        nop = mybir.InstNoOp(
                            name=f"{ins.name}-w{j}", ins=[], outs=[],
                            engine=ins.engine,
                            sync_info=mybir.SyncInfo(on_wait=[w], on_update=[]))
                        newl.append(nop)
                        n_split += 1
                    ins.sync_info = mybir.SyncInfo(on_wait=waits[:max_waits],
                                                   on_update=si.on_update)
                newl.append(ins)
            bb.instructions = newl
    return n_split


def _get_program(TL, H, W, reps=1):
    key = (TL, H, W, reps)
    if key not in _BUILD_CACHE:
        _BUILD_CACHE[key] = _build_program(TL, H, W, reps=reps)
    return _BUILD_CACHE[key]


def kernel(segmentation, num_instances=None, **_ignored):
    from concourse.bass_utils import run_bass_kernel_spmd

    seg = np.asarray(segmentation)
    T, H, W = seg.shape
    assert T % _NCORES == 0
    TL = T // _NCORES
    nc, tables = _get_program(TL, H, W)

    seg = np.ascontiguousarray(seg, dtype=np.int32)
    in_maps = [{"seg": seg[i * TL:(i + 1) * TL], **tables}
               for i in range(_NCORES)]
    res = run_bass_kernel_spmd(nc, in_maps, list(range(_NCORES)))
    out = np.concatenate([res.results[i]["boxes"] for i in range(_NCORES)], axis=0)
    return out.astype(np.float32)


# revision 21
# speedup vs baseline: 1.6792x; 1.1468x over previous
"""Trainium2 Bass kernel for nn_BoxesFromMasks (per-frame segment bounding boxes).

Algorithm (per core, data-parallel over frames, TL=2 frames/core):
  Build per-pixel 64-bit one-hot bitmasks (2 u32 planes) of the instance id via
  the exponent-bit trick (ACT builds the f32 bit pattern of 2^k as an int, a
  second ACT converts value->u32, truncating out-of-range ids to 0):
    lo plane: id s in [0,32)  -> bit (31-s)
    hi plane: id s in [32,64) -> bit (s-32)
  Row masks:  OR-tree each 128-row chunk along columns (DVE), 16-wide leftovers
              folded once at extraction time.
  Col masks:  OR-accumulate chunks into acc[128,2,W]; pre-fold partitions
              128->64; DMA-transpose (u16); OR-tree the 64 contributors.
  Extraction (batched, no DRAM bounce): 16 u16 shift ops expand bits to
  E-tables, constant value-tables select coordinates via i16 mult/add, strided
  tensor_reduce min/max, one 3x128 transpose fold, and negative-stride output
  DMAs undo the bit-order permutation.
"""

import numpy as np

_T, _H, _W, _N = 16, 1024, 2048, 64
_NCORES = 8

_BUILD_CACHE = {}


def _build_program(TL, H, W, split_waits=True, reps=1):
    from contextlib import ExitStack

    import bass_rust
    import concourse.bass as bass
    import concourse.tile as tile
    import concourse.mybir as mybir
    from concourse.alu_op_type import AluOpType as Op

    f32 = mybir.dt.float32
    i32 = mybir.dt.int32
    u32 = mybir.dt.uint32
    u16 = mybir.dt.uint16
    i16 = mybir.dt.int16
    Copy = mybir.ActivationFunctionType.Copy
    X = mybir.AxisListType.X

    P = 128
    CH = H // P                   # row chunks per frame (8)
    KT = 4                        # transpose calls per frame (each 2048 u16 cols)
    MPER = 16                     # 128-col blocks per transpose call
    NSEG = 2                      # seg DMA splits per chunk
    LFT = 32                      # row-OR tree leftover width
    BIG = 32767
    assert TL == 2 and CH == 8 and W == 2048

    # ---- constant value tables (i16) ----
    pp = np.arange(P)
    # Y: value v(p, c) = 128c + p ; table shape [P, 64(pl f j), CH, 2h]
    yv = (128 * np.arange(CH)[None, :] + pp[:, None]).astype(np.int64)   # [P, CH]
    ty_mb = np.broadcast_to((yv - BIG)[:, None, :, None],
                            (P, 64, CH, 2)).astype(np.int16)
    ty_p1 = np.broadcast_to((yv + 1)[:, None, :, None],
                            (P, 64, CH, 2)).astype(np.int16)
    # X: value v(q, klo, m) = klo*1024 + 64m + (q>>1) ; table [P, 64(pl f j), 32(klo m)]
    klo = np.arange(2)
    mm = np.arange(MPER)
    xv = ((klo[:, None] * 1024 + 64 * mm[None, :]).reshape(-1)[None, :]
          + (pp[:, None] // 2)).astype(np.int64)                         # [P, 32]
    tx_mb = np.broadcast_to((xv - BIG)[:, None, :], (P, 64, 32)).astype(np.int16)
    tx_p1 = np.broadcast_to((xv + 1)[:, None, :], (P, 64, 32)).astype(np.int16)

    tables = {"ty_mb": ty_mb, "ty_p1": ty_p1, "tx_mb": tx_mb, "tx_p1": tx_p1}

    nc = bass.Bass()
    seg_in = nc.dram_tensor("seg", [TL, H, W], i32, kind="ExternalInput")
    boxes_out = nc.dram_tensor("boxes", [TL, 64, 4], f32, kind="ExternalOutput")
    d_tabs = {n: nc.dram_tensor(n, list(t.shape), i16, kind="ExternalInput")
              for n, t in tables.items()}

    def dram_ap(t, offset_elems, dims):
        """Manual DRAM AP: dims = [(stride_elems, count), ...]."""
        a2 = t[:].copy()
        a2.offset = offset_elems
        a2.ap = bass_rust.VecI64Pair([[s, n] for s, n in dims])
        return a2

    with tile.TileContext(nc) as tc, ExitStack() as ctx:
        constp = ctx.enter_context(tc.tile_pool(name="consts", bufs=1))
        segp = ctx.enter_context(tc.tile_pool(name="segp", bufs=3))
        ep = ctx.enter_context(tc.tile_pool(name="ep", bufs=3))
        accp = ctx.enter_context(tc.tile_pool(name="accp", bufs=2))
        accTp = ctx.enter_context(tc.tile_pool(name="accTp", bufs=2))
        rmp = ctx.enter_context(tc.tile_pool(name="rmp", bufs=1))
        trp = ctx.enter_context(tc.tile_pool(name="trp", bufs=2))
        xp = ctx.enter_context(tc.tile_pool(name="xp", bufs=1))
        smallp = ctx.enter_context(tc.tile_pool(name="smallp", bufs=1))

        c_ty_mb = constp.tile([P, 64, CH, 2], i16)
        c_ty_p1 = constp.tile([P, 64, CH, 2], i16)
        c_tx_mb = constp.tile([P, 64, 32], i16)
        c_tx_p1 = constp.tile([P, 64, 32], i16)
        const_loaded = [False]

        def load_consts():
            if const_loaded[0]:
                return
            const_loaded[0] = True
            for t, n in [(c_ty_mb, "ty_mb"), (c_ty_p1, "ty_p1"),
                         (c_tx_mb, "tx_mb"), (c_tx_p1, "tx_p1")]:
                nc.scalar.dma_start(t[:], d_tabs[n][:])

        for _rep in range(reps):
            # rmask16: [p, pl, f, c, LFT] u32 (pl-major for contiguous planes)
            rmask16 = rmp.tile([P, 2, TL, CH, LFT], u32, tag="rmask16")
            # CMX: [q, pl, f, klo, m] u16 (compacted column masks)
            CMX = xp.tile([P, 2, TL, 2, MPER], u16, tag="cmx")

            # ================= main loop =================
            # seg tiles are created and their loads issued ahead of use so
            # next-frame loads precede this frame's transposes on the queue
            seg_tiles = {}

            def issue_seg(f, c):
                if f >= TL or (f, c) in seg_tiles:
                    return
                s = segp.tile([P, W], i32, tag="seg")
                nseg = 4 if (f == 0 and c == 0) else NSEG
                rows = P // nseg
                for k in range(nseg):
                    nc.sync.dma_start(
                        s[rows * k:rows * (k + 1), :],
                        seg_in[f, c * P + rows * k:c * P + rows * (k + 1), :])
                seg_tiles[(f, c)] = s

            for c2 in range(3):
                issue_seg(0, c2)
            load_consts()

            for f in range(TL):
                acc = accp.tile([P, 2, W], u32)
                prev_u = None
                for c in range(CH):
                    issue_seg(f, c)
                    s = seg_tiles.pop((f, c))

                    e = ep.tile([P, 2, W], i32)
                    # lo: bitpattern of 2^(31-s) = (158-s)<<23 ; hi: 2^(s-32) = (s+95)<<23
                    # first chunk of the kernel: split build/cast into partition
                    # halves so the pipeline fills ~4us sooner
                    halves = ([(0, 64), (64, 128)] if (f == 0 and c == 0)
                              else [(0, 128)])
                    for p0, p1 in halves:
                        nc.scalar.activation(e[p0:p1, 0, :], s[p0:p1], Copy,
                                             bias=1325400064.0, scale=-8388608.0)
                        nc.gpsimd.tensor_scalar(e[p0:p1, 1, :], s[p0:p1],
                                                8388608, 796917760,
                                                Op.mult, Op.add)
                        nc.scalar.activation(e[p0:p1].bitcast(u32),
                                             e[p0:p1].bitcast(f32), Copy)
                    u = e[:].bitcast(u32)  # cast in place

                    # column accumulate (DVE; only DVE has integer bitwise ops)
                    if c == 0:
                        prev_u = u
                    elif c == 1:
                        nc.vector.tensor_tensor(acc[:], u, prev_u, Op.bitwise_or)
                    else:
                        nc.vector.tensor_tensor(acc[:], u, acc[:], Op.bitwise_or)

                    # row masks: OR-tree along columns (DVE). In place, except
                    # chunk 0 whose u must stay intact for the c==1 accumulate.
                    if c == 0:
                        tr0 = trp.tile([P, 2, W // 2], u32, tag="tr0")
                        base = tr0[:]
                    else:
                        base = e[:, :, 0:W // 2].bitcast(u32)
                    w = W // 2
                    nc.vector.tensor_tensor(base[:, :, 0:w], u[:, :, 0:w],
                                            u[:, :, w:2 * w], Op.bitwise_or)
                    w //= 2
                    while w > LFT:
                        nc.vector.tensor_tensor(base[:, :, 0:w], base[:, :, 0:w],
                                                base[:, :, w:2 * w], Op.bitwise_or)
                        w //= 2
                    # last level writes the leftovers straight into rmask16
                    nc.vector.tensor_tensor(rmask16[:, :, f, c, :],
                                            base[:, :, 0:LFT],
                                            base[:, :, LFT:2 * LFT], Op.bitwise_or)

                # prefetch next frame's first chunks before the transposes so
                # their loads aren't queued behind acc-dependent triggers
                for c2 in range(3):
                    issue_seg(f + 1, c2)

                # ---- frame tail: transpose (u16), fold the 128 contributors
                accT = accTp.tile([P, KT, MPER, P], u16, tag="accT")
                a16 = acc[:].bitcast(u16).rearrange("p a b -> p (a b)")
                for k in range(KT):
                    nc.sync.dma_start(accT[:, k],
                                      a16[:, 2048 * k:2048 * (k + 1)],
                                      transpose=True)
                w = 64
                while w >= 1:
                    nc.vector.tensor_tensor(accT[:, :, :, 0:w],
                                            accT[:, :, :, 0:w],
                                            accT[:, :, :, w:2 * w],
                                            Op.bitwise_or)
                    w //= 2
                # compact: CMX[q, pl, f, klo, m] <- accT[q, (pl,klo), m, 0]
                nc.vector.tensor_copy(
                    CMX[:, :, f, :, :],
                    accT[:, :, :, 0].rearrange("q (pl klo) m -> q pl klo m",
                                               pl=2, klo=2))

            # ================= extraction =================
            # fold rowmask leftovers [..., LFT] -> [..., 1]
            rmf = rmask16[:].rearrange("p pl f c w -> p (pl f c) w")
            w = LFT // 2
            while w >= 1:
                nc.vector.tensor_tensor(rmf[:, :, 0:w], rmf[:, :, 0:w],
                                        rmf[:, :, w:2 * w], Op.bitwise_or)
                w //= 2

            # --- Y side ---
            # rm u16 view: [p, pl, f, c, h]  (h = u16 half; bit b32 = 16h + j)
            rmv = rmask16[:].bitcast(u16)[:, :, :, :, 0:2]
            rm_e = rmv.rearrange("p pl f c h -> p (pl f) c h")
            Ey = xp.tile([P, 2, TL, 16, CH, 2], i16, tag="ey")
            for j in range(16):
                nc.vector.tensor_scalar(
                    Ey[:, :, :, j].rearrange("p pl f c h -> p (pl f) c h").bitcast(u16),
                    rm_e, j, 1, Op.logical_shift_right, Op.bitwise_and)

            ey_flat = Ey[:].rearrange("p pl f j c h -> p (pl f j) c h")
            CY = xp.tile([P, 64, CH, 2], i16, tag="cy")
            S = smallp.tile([P, 384], i16)
            # Y block layout: col = t*128 + h*64 + (pl f j)  (h-major for output DMAs)
            Sy = S[:, 0:256].rearrange("p (t h a) -> p t h a", t=2, h=2, a=64)
            # ymin: min over c of E*(v-BIG)+BIG
            nc.vector.tensor_tensor(CY[:], ey_flat, c_ty_mb[:], Op.mult)
            nc.vector.tensor_scalar(CY[:], CY[:], BIG, None, Op.add)
            nc.vector.tensor_reduce(Sy[:, 0], CY[:].rearrange("p a c h -> p h a c"),
                                    axis=X, op=Op.min)
            # ymax(+1): max over c of E*(v+1)
            nc.vector.tensor_tensor(CY[:], ey_flat, c_ty_p1[:], Op.mult)
            nc.vector.tensor_reduce(Sy[:, 1], CY[:].rearrange("p a c h -> p h a c"),
                                    axis=X, op=Op.max)

            # --- X side ---
            cmx_flat = CMX[:].rearrange("q pl f klo m -> q (pl f) (klo m)")
            Ex = xp.tile([P, 4, 16, 32], i16, tag="ex")
            for j in range(16):
                nc.vector.tensor_scalar(Ex[:, :, j].bitcast(u16), cmx_flat,
                                        j, 1, Op.logical_shift_right, Op.bitwise_and)
            ex_flat = Ex[:].rearrange("q a j km -> q (a j) km")
            CXt = xp.tile([P, 64, 32], i16, tag="cx")
            nc.vector.tensor_tensor(CXt[:], ex_flat, c_tx_mb[:], Op.mult)
            nc.vector.tensor_scalar(CXt[:], CXt[:], BIG, None, Op.add)
            nc.vector.tensor_reduce(S[:, 256:320], CXt[:], axis=X, op=Op.min)
            nc.vector.tensor_tensor(CXt[:], ex_flat, c_tx_p1[:], Op.mult)
            nc.vector.tensor_reduce(S[:, 320:384], CXt[:], axis=X, op=Op.max)

            # --- partition fold: 3 transposes + reduces ---
            ST = smallp.tile([P, 3, 128], i16)
            for t in range(3):
                eng = nc.scalar if t % 2 else nc.sync
                eng.dma_start(ST[:, t], S[:, 128 * t:128 * (t + 1)],
                              transpose=True)
            # Y: rows (pl f j h); reduce over all 128 contributors
            FY = smallp.tile([P, 2], i16)
            nc.vector.tensor_reduce(FY[:, 0:1], ST[:, 0], axis=X, op=Op.min)
            nc.vector.tensor_reduce(FY[:, 1:2], ST[:, 1], axis=X, op=Op.max)
            # X: rows 0:64 = min (pl f j), 64:128 = max; contributors split by
            # parity g = q&1 (halfword h2 = g); output cols = g
            FX = smallp.tile([P, 2], i16)
            nc.vector.tensor_reduce(
                FX[0:64, :], ST[0:64, 2].rearrange("p (x g) -> p g x", g=2),
                axis=X, op=Op.min)
            nc.vector.tensor_reduce(
                FX[64:128, :], ST[64:128, 2].rearrange("p (x g) -> p g x", g=2),
                axis=X, op=Op.max)

            # --- fixups in f32 ---
            # mins: v==BIG (absent) -> 2147483648.0 ; maxes: v-1 == -1 -> -2^31
            BY = smallp.tile([P, 2], f32)
            BX = smallp.tile([P, 2], f32)
            fy = smallp.tile([P, 2], f32)
            fx = smallp.tile([P, 2], f32)
            nc.vector.tensor_copy(BY[:], FY[:])
            nc.vector.tensor_copy(BX[:], FX[:])
            nc.vector.tensor_scalar(BY[:, 1:2], BY[:, 1:2], 1, 0, Op.subtract, Op.add)
            nc.vector.tensor_scalar(BX[64:128, :], BX[64:128, :], 1, 0,
                                    Op.subtract, Op.add)
            nc.vector.tensor_scalar(fy[:, 0:1], BY[:, 0:1], 32767.0, 2147450880.0,
                                    Op.is_equal, Op.mult)
            nc.vector.tensor_scalar(fy[:, 1:2], BY[:, 1:2], -1.0, -2147483647.0,
                                    Op.is_equal, Op.mult)
            nc.vector.tensor_scalar(fx[0:64, :], BX[0:64, :], 32767.0, 2147450880.0,
                                    Op.is_equal, Op.mult)
            nc.vector.tensor_scalar(fx[64:128, :], BX[64:128, :], -1.0, -2147483647.0,
                                    Op.is_equal, Op.mult)
            nc.vector.tensor_tensor(BY[:], BY[:], fy[:], Op.add)
            nc.vector.tensor_tensor(BX[:], BX[:], fx[:], Op.add)

            # --- output DMAs ---
            # boxes[f, n, k]: k: 0 xmin, 1 ymin, 2 xmax, 3 ymax
            # Y rows p = h*64 + (pl*2+f)*16 + j ; n: pl=0: 31-16h-j, pl=1: 32+16h+j
            outn = [0]

            def out_dma(dst, src):
                eng = nc.scalar if outn[0] % 2 else nc.sync
                outn[0] += 1
                eng.dma_start(dst, src)

            for col, k in [(0, 1), (1, 3)]:
                for h in range(2):
                    base = h * 64
                    out_dma(dram_ap(boxes_out, k + 4 * (31 - 16 * h),
                                    [(256, TL), (-4, 16)]),
                            BY[base:base + 32, col:col + 1])
                    out_dma(dram_ap(boxes_out, k + 4 * (32 + 16 * h),
                                    [(256, TL), (4, 16)]),
                            BY[base + 32:base + 64, col:col + 1])
            # X rows p = base + (pl*2+f)*16+j ; n: pl=0: 31-16g-j, pl=1: 32+16g+j
            for base, k in [(0, 0), (64, 2)]:
                for g in range(2):
                    out_dma(dram_ap(boxes_out, k + 4 * (31 - 16 * g),
                                    [(256, TL), (-4, 16)]),
                            BX[base:base + 32, g:g + 1])
                    out_dma(dram_ap(boxes_out, k + 4 * (32 + 16 * g),
                                    [(256, TL), (4, 16)]),
                            BX[base + 32:base + 64, g:g + 1])

    nc.finalize()
    if split_waits:
        _split_excess_waits(nc, mybir)
    return nc, tables


def _split_excess_waits(nc, mybir):
    """Hoist extra sem waits onto preceding NoOps.

    This walrus build rejects instructions carrying more sync-wait
    conditions than their ISA encoding holds (1 for TPB_CTRL ops and for
    Pool/core_v2 compute ops; 2 elsewhere, conservatively). Semantics are
    identical with the waits split onto dedicated NoOps just before the
    instruction.
    """
    n_split = 0
    for f in nc.m.functions:
        for bb in f.blocks:
            newl = []
            for ins in bb.instructions:
                si = ins.sync_info
                max_waits = 1
                if si and si.on_wait and len(si.on_wait) > max_waits:
                    waits = list(si.on_wait)
                    for j, w in enumerate(waits[max_waits:]):
                        nop = mybir.InstNoOp(
                            name=f"{ins.name}-w{j}", ins=[], outs=[],
                            engine=ins.engine,
                            sync_info=mybir.SyncInfo(on_wait=[w], on_update=[]))
                        newl.append(nop)
                        n_split += 1
                    ins.sync_info = mybir.SyncInfo(on_wait=waits[:max_waits],
                                                   on_update=si.on_update)
                newl.append(ins)
            bb.instructions = newl
    return n_split


def _get_program(TL, H, W, reps=1):
    key = (TL, H, W, reps)
    if key not in _BUILD_CACHE:
        _BUILD_CACHE[key] = _build_program(TL, H, W, reps=reps)
    return _BUILD_CACHE[key]


def kernel(segmentation, num_instances=None, **_ignored):
    from concourse.bass_utils import run_bass_kernel_spmd

    seg = np.asarray(segmentation)
    T, H, W = seg.shape
    assert T % _NCORES == 0
    TL = T // _NCORES
    nc, tables = _get_program(TL, H, W)

    seg = np.ascontiguousarray(seg, dtype=np.int32)
    in_maps = [{"seg": seg[i * TL:(i + 1) * TL], **tables}
               for i in range(_NCORES)]
    res = run_bass_kernel_spmd(nc, in_maps, list(range(_NCORES)))
    out = np.concatenate([res.results[i]["boxes"] for i in range(_NCORES)], axis=0)
    return out.astype(np.float32)


# revision 25
# speedup vs baseline: 1.8034x; 1.0740x over previous
"""Trainium2 Bass kernel for nn_BoxesFromMasks (per-frame segment bounding boxes).

Algorithm (per core, data-parallel over frames, TL=2 frames/core):
  Build per-pixel 64-bit one-hot bitmasks (2 u32 planes) of the instance id via
  the exponent-bit trick (ACT builds the f32 bit pattern of 2^k as an int, a
  second ACT converts value->u32, truncating out-of-range ids to 0):
    lo plane: id s in [0,32)  -> bit (31-s)
    hi plane: id s in [32,64) -> bit (s-32)
  Row masks:  OR-tree each 128-row chunk along columns (DVE), 16-wide leftovers
              folded once at extraction time.
  Col masks:  OR-accumulate chunks into acc[128,2,W]; pre-fold partitions
              128->64; DMA-transpose (u16); OR-tree the 64 contributors.
  Extraction (batched, no DRAM bounce): 16 u16 shift ops expand bits to
  E-tables, constant value-tables select coordinates via i16 mult/add, strided
  tensor_reduce min/max, one 3x128 transpose fold, and negative-stride output
  DMAs undo the bit-order permutation.
"""

import numpy as np

_T, _H, _W, _N = 16, 1024, 2048, 64
_NCORES = 8

_BUILD_CACHE = {}


def _build_program(TL, H, W, split_waits=True, reps=1):
    from contextlib import ExitStack

    import bass_rust
    import concourse.bass as bass
    import concourse.tile as tile
    import concourse.mybir as mybir
    from concourse.alu_op_type import AluOpType as Op

    f32 = mybir.dt.float32
    i32 = mybir.dt.int32
    u32 = mybir.dt.uint32
    u16 = mybir.dt.uint16
    i16 = mybir.dt.int16
    Copy = mybir.ActivationFunctionType.Copy
    X = mybir.AxisListType.X

    P = 128
    CH = H // P                   # row chunks per frame (8)
    KT = 4                        # transpose calls per frame (each 2048 u16 cols)
    MPER = 16                     # 128-col blocks per transpose call
    NSEG = 2                      # seg DMA splits per chunk
    BIG = 32767
    assert TL == 2 and CH == 8 and W == 2048

    # ---- constant value tables (i16) ----
    pp = np.arange(P)
    # Y: value v(p, c) = 128c + p ; table shape [P, 64(pl f j), CH, 2h]
    yv = (128 * np.arange(CH)[None, :] + pp[:, None]).astype(np.int64)   # [P, CH]
    ty_mb = np.broadcast_to((yv - BIG)[:, None, :, None],
                            (P, 64, CH, 2)).astype(np.int16)
    ty_p1 = np.broadcast_to((yv + 1)[:, None, :, None],
                            (P, 64, CH, 2)).astype(np.int16)
    # X: value v(q, klo, m) = klo*1024 + 64m + (q>>1) ; table [P, 64(pl f j), 32(klo m)]
    klo = np.arange(2)
    mm = np.arange(MPER)
    xv = ((klo[:, None] * 1024 + 64 * mm[None, :]).reshape(-1)[None, :]
          + (pp[:, None] // 2)).astype(np.int64)                         # [P, 32]
    tx_mb = np.broadcast_to((xv - BIG)[:, None, :], (P, 64, 32)).astype(np.int16)
    tx_p1 = np.broadcast_to((xv + 1)[:, None, :], (P, 64, 32)).astype(np.int16)

    tables = {"ty_mb": ty_mb, "ty_p1": ty_p1, "tx_mb": tx_mb, "tx_p1": tx_p1}

    nc = bass.Bass()
    seg_in = nc.dram_tensor("seg", [TL, H, W], i32, kind="ExternalInput")
    boxes_out = nc.dram_tensor("boxes", [TL, 64, 4], f32, kind="ExternalOutput")
    d_tabs = {n: nc.dram_tensor(n, list(t.shape), i16, kind="ExternalInput")
              for n, t in tables.items()}

    def dram_ap(t, offset_elems, dims):
        """Manual DRAM AP: dims = [(stride_elems, count), ...]."""
        a2 = t[:].copy()
        a2.offset = offset_elems
        a2.ap = bass_rust.VecI64Pair([[s, n] for s, n in dims])
        return a2

    with tile.TileContext(nc) as tc, ExitStack() as ctx:
        constp = ctx.enter_context(tc.tile_pool(name="consts", bufs=1))
        segp = ctx.enter_context(tc.tile_pool(name="segp", bufs=3))
        ep = ctx.enter_context(tc.tile_pool(name="ep", bufs=3))
        accp = ctx.enter_context(tc.tile_pool(name="accp", bufs=2))
        accTp = ctx.enter_context(tc.tile_pool(name="accTp", bufs=2))
        rmp = ctx.enter_context(tc.tile_pool(name="rmp", bufs=1))
        xp = ctx.enter_context(tc.tile_pool(name="xp", bufs=1))
        smallp = ctx.enter_context(tc.tile_pool(name="smallp", bufs=1))

        c_ty_mb = constp.tile([P, 64, CH, 2], i16)
        c_ty_p1 = constp.tile([P, 64, CH, 2], i16)
        c_tx_mb = constp.tile([P, 64, 32], i16)
        c_tx_p1 = constp.tile([P, 64, 32], i16)
        const_loaded = [False]

        def load_consts():
            if const_loaded[0]:
                return
            const_loaded[0] = True
            for t, n in [(c_ty_mb, "ty_mb"), (c_ty_p1, "ty_p1"),
                         (c_tx_mb, "tx_mb"), (c_tx_p1, "tx_p1")]:
                nc.scalar.dma_start(t[:], d_tabs[n][:])

        for _rep in range(reps):
            # rmask: [p, pl, f, c] u32 (pl-major for contiguous planes)
            rmask = rmp.tile([P, 2, TL, CH], u32, tag="rmask")
            # CMX: [q, pl, f, klo, m] u16 (compacted column masks)
            CMX = xp.tile([P, 2, TL, 2, MPER], u16, tag="cmx")

            # ================= main loop =================
            # seg tiles are created and their loads issued ahead of use so
            # next-frame loads precede this frame's transposes on the queue
            seg_tiles = {}

            def issue_seg(f, c):
                if f >= TL or (f, c) in seg_tiles:
                    return
                s = segp.tile([P, W], i32, tag="seg")
                nseg = 4 if (f == 0 and c == 0) else NSEG
                rows = P // nseg
                for k in range(nseg):
                    nc.sync.dma_start(
                        s[rows * k:rows * (k + 1), :],
                        seg_in[f, c * P + rows * k:c * P + rows * (k + 1), :])
                seg_tiles[(f, c)] = s

            for c2 in range(3):
                issue_seg(0, c2)
            load_consts()

            for f in range(TL):
                acc = accp.tile([P, 2, W], u32)
                prev_u = None
                for c in range(CH):
                    issue_seg(f, c)
                    s = seg_tiles.pop((f, c))

                    e = ep.tile([P, 2, W], i32)
                    # lo: bitpattern of 2^(31-s) = (158-s)<<23 ; hi: 2^(s-32) = (s+95)<<23
                    # first chunk of the kernel: split build/cast into partition
                    # halves so the pipeline fills ~4us sooner
                    halves = ([(0, 64), (64, 128)] if (f == 0 and c == 0)
                              else [(0, 128)])
                    for p0, p1 in halves:
                        nc.scalar.activation(e[p0:p1, 0, :], s[p0:p1], Copy,
                                             bias=1325400064.0, scale=-8388608.0)
                        nc.gpsimd.tensor_scalar(e[p0:p1, 1, :], s[p0:p1],
                                                8388608, 796917760,
                                                Op.mult, Op.add)
                        nc.scalar.activation(e[p0:p1].bitcast(u32),
                                             e[p0:p1].bitcast(f32), Copy)
                    u = e[:].bitcast(u32)  # cast in place

                    # column accumulate (DVE; only DVE has integer bitwise ops)
                    if c == 0:
                        prev_u = u
                    elif c == 1:
                        nc.vector.tensor_tensor(acc[:], u, prev_u, Op.bitwise_or)
                    else:
                        nc.vector.tensor_tensor(acc[:], u, acc[:], Op.bitwise_or)

                    # row masks: single OR-reduce along columns (DVE)
                    nc.vector.tensor_reduce(rmask[:, :, f, c], u, axis=X,
                                            op=Op.bitwise_or)

                # prefetch next frame's first chunks before the transposes so
                # their loads aren't queued behind acc-dependent triggers
                for c2 in range(3):
                    issue_seg(f + 1, c2)

                # ---- frame tail: transpose (u16), fold the 128 contributors
                accT = accTp.tile([P, KT, MPER, P], u16, tag="accT")
                a16 = acc[:].bitcast(u16).rearrange("p a b -> p (a b)")
                for k in range(KT):
                    nc.sync.dma_start(accT[:, k],
                                      a16[:, 2048 * k:2048 * (k + 1)],
                                      transpose=True)
                w = 64
                while w >= 1:
                    nc.vector.tensor_tensor(accT[:, :, :, 0:w],
                                            accT[:, :, :, 0:w],
                                            accT[:, :, :, w:2 * w],
                                            Op.bitwise_or)
                    w //= 2
                # compact: CMX[q, pl, f, klo, m] <- accT[q, (pl,klo), m, 0]
                nc.vector.tensor_copy(
                    CMX[:, :, f, :, :],
                    accT[:, :, :, 0].rearrange("q (pl klo) m -> q pl klo m",
                                               pl=2, klo=2))

            # ================= extraction =================
            # --- Y side ---
            # rm u16 view: [p, pl, f, c, h]  (h = u16 half; bit b32 = 16h + j)
            rm_e = rmask[:].bitcast(u16).rearrange(
                "p pl f (c h) -> p (pl f) c h", c=CH, h=2)
            Ey = xp.tile([P, 2, TL, 16, CH, 2], i16, tag="ey")
            for j in range(16):
                nc.vector.tensor_scalar(
                    Ey[:, :, :, j].rearrange("p pl f c h -> p (pl f) c h").bitcast(u16),
                    rm_e, j, 1, Op.logical_shift_right, Op.bitwise_and)

            ey_flat = Ey[:].rearrange("p pl f j c h -> p (pl f j) c h")
            CY = xp.tile([P, 64, CH, 2], i16, tag="cy")
            S = smallp.tile([P, 384], i16)
            # Y block layout: col = t*128 + h*64 + (pl f j)  (h-major for output DMAs)
            Sy = S[:, 0:256].rearrange("p (t h a) -> p t h a", t=2, h=2, a=64)
            # ymin: min over c of E*(v-BIG)+BIG
            nc.vector.tensor_tensor(CY[:], ey_flat, c_ty_mb[:], Op.mult)
            nc.vector.tensor_scalar(CY[:], CY[:], BIG, None, Op.add)
            nc.vector.tensor_reduce(Sy[:, 0], CY[:].rearrange("p a c h -> p h a c"),
                                    axis=X, op=Op.min)
            # ymax(+1): max over c of E*(v+1)
            nc.vector.tensor_tensor(CY[:], ey_flat, c_ty_p1[:], Op.mult)
            nc.vector.tensor_reduce(Sy[:, 1], CY[:].rearrange("p a c h -> p h a c"),
                                    axis=X, op=Op.max)

            # --- X side ---
            cmx_flat = CMX[:].rearrange("q pl f klo m -> q (pl f) (klo m)")
            Ex = xp.tile([P, 4, 16, 32], i16, tag="ex")
            for j in range(16):
                nc.vector.tensor_scalar(Ex[:, :, j].bitcast(u16), cmx_flat,
                                        j, 1, Op.logical_shift_right, Op.bitwise_and)
            ex_flat = Ex[:].rearrange("q a j km -> q (a j) km")
            CXt = xp.tile([P, 64, 32], i16, tag="cx")
            nc.vector.tensor_tensor(CXt[:], ex_flat, c_tx_mb[:], Op.mult)
            nc.vector.tensor_scalar(CXt[:], CXt[:], BIG, None, Op.add)
            nc.vector.tensor_reduce(S[:, 256:320], CXt[:], axis=X, op=Op.min)
            nc.vector.tensor_tensor(CXt[:], ex_flat, c_tx_p1[:], Op.mult)
            nc.vector.tensor_reduce(S[:, 320:384], CXt[:], axis=X, op=Op.max)

            # --- partition fold: 3 transposes + reduces ---
            ST = smallp.tile([P, 3, 128], i16)
            for t in range(3):
                eng = nc.scalar if t % 2 else nc.sync
                eng.dma_start(ST[:, t], S[:, 128 * t:128 * (t + 1)],
                              transpose=True)
            # Y: rows (pl f j h); reduce over all 128 contributors
            FY = smallp.tile([P, 2], i16)
            nc.vector.tensor_reduce(FY[:, 0:1], ST[:, 0], axis=X, op=Op.min)
            nc.vector.tensor_reduce(FY[:, 1:2], ST[:, 1], axis=X, op=Op.max)
            # X: rows 0:64 = min (pl f j), 64:128 = max; contributors split by
            # parity g = q&1 (halfword h2 = g); output cols = g
            FX = smallp.tile([P, 2], i16)
            nc.vector.tensor_reduce(
                FX[0:64, :], ST[0:64, 2].rearrange("p (x g) -> p g x", g=2),
                axis=X, op=Op.min)
            nc.vector.tensor_reduce(
                FX[64:128, :], ST[64:128, 2].rearrange("p (x g) -> p g x", g=2),
                axis=X, op=Op.max)

            # --- fixups in f32 ---
            # mins: v==BIG (absent) -> 2147483648.0 ; maxes: v-1 == -1 -> -2^31
            BY = smallp.tile([P, 2], f32)
            BX = smallp.tile([P, 2], f32)
            fy = smallp.tile([P, 2], f32)
            fx = smallp.tile([P, 2], f32)
            nc.vector.tensor_copy(BY[:], FY[:])
            nc.vector.tensor_copy(BX[:], FX[:])
            nc.vector.tensor_scalar(BY[:, 1:2], BY[:, 1:2], 1, 0, Op.subtract, Op.add)
            nc.vector.tensor_scalar(BX[64:128, :], BX[64:128, :], 1, 0,
                                    Op.subtract, Op.add)
            nc.vector.tensor_scalar(fy[:, 0:1], BY[:, 0:1], 32767.0, 2147450880.0,
                                    Op.is_equal, Op.mult)
            nc.vector.tensor_scalar(fy[:, 1:2], BY[:, 1:2], -1.0, -2147483647.0,
                                    Op.is_equal, Op.mult)
            nc.vector.tensor_scalar(fx[0:64, :], BX[0:64, :], 32767.0, 2147450880.0,
                                    Op.is_equal, Op.mult)
            nc.vector.tensor_scalar(fx[64:128, :], BX[64:128, :], -1.0, -2147483647.0,
                                    Op.is_equal, Op.mult)
            nc.vector.tensor_tensor(BY[:], BY[:], fy[:], Op.add)
            nc.vector.tensor_tensor(BX[:], BX[:], fx[:], Op.add)

            # --- output DMAs ---
            # boxes[f, n, k]: k: 0 xmin, 1 ymin, 2 xmax, 3 ymax
            # Y rows p = h*64 + (pl*2+f)*16 + j ; n: pl=0: 31-16h-j, pl=1: 32+16h+j
            outn = [0]

            def out_dma(dst, src):
                eng = nc.scalar if outn[0] % 2 else nc.sync
                outn[0] += 1
                eng.dma_start(dst, src)

            for col, k in [(0, 1), (1, 3)]:
                for h in range(2):
                    base = h * 64
                    out_dma(dram_ap(boxes_out, k + 4 * (31 - 16 * h),
                                    [(256, TL), (-4, 16)]),
                            BY[base:base + 32, col:col + 1])
                    out_dma(dram_ap(boxes_out, k + 4 * (32 + 16 * h),
                                    [(256, TL), (4, 16)]),
                            BY[base + 32:base + 64, col:col + 1])
            # X rows p = base + (pl*2+f)*16+j ; n: pl=0: 31-16g-j, pl=1: 32+16g+j
            for base, k in [(0, 0), (64, 2)]:
                for g in range(2):
                    out_dma(dram_ap(boxes_out, k + 4 * (31 - 16 * g),
                                    [(256, TL), (-4, 16)]),
                            BX[base:base + 32, g:g + 1])
                    out_dma(dram_ap(boxes_out, k + 4 * (32 + 16 * g),
                                    [(256, TL), (4, 16)]),
                            BX[base + 32:base + 64, g:g + 1])

    nc.finalize()
    if split_waits:
        _split_excess_waits(nc, mybir)
    return nc, tables


def _split_excess_waits(nc, mybir):
    """Hoist extra sem waits onto preceding NoOps.

    This walrus build rejects instructions carrying more sync-wait
    conditions than their ISA encoding holds (1 for TPB_CTRL ops and for
    Pool/core_v2 compute ops; 2 elsewhere, conservatively). Semantics are
    identical with the waits split onto dedicated NoOps just before the
    instruction.
    """
    n_split = 0
    for f in nc.m.functions:
        for bb in f.blocks:
            newl = []
            for ins in bb.instructions:
                si = ins.sync_info
                max_waits = 1
                if si and si.on_wait and len(si.on_wait) > max_waits:
                    waits = list(si.on_wait)
                    for j, w in enumerate(waits[max_waits:]):
                        nop = mybir.InstNoOp(
                            name=f"{ins.name}-w{j}", ins=[], outs=[],
                            engine=ins.engine,
                            sync_info=mybir.SyncInfo(on_wait=[w], on_update=[]))
                        newl.append(nop)
                        n_split += 1
                    ins.sync_info = mybir.SyncInfo(on_wait=waits[:max_waits],
                                                   on_update=si.on_update)
                newl.append(ins)
            bb.instructions = newl
    return n_split


def _get_program(TL, H, W, reps=1):
    key = (TL, H, W, reps)
    if key not in _BUILD_CACHE:
        _BUILD_CACHE[key] = _build_program(TL, H, W, reps=reps)
    return _BUILD_CACHE[key]


def kernel(segmentation, num_instances=None, **_ignored):
    from concourse.bass_utils import run_bass_kernel_spmd

    seg = np.asarray(segmentation)
    T, H, W = seg.shape
    assert T % _NCORES == 0
    TL = T // _NCORES
    nc, tables = _get_program(TL, H, W)

    seg = np.ascontiguousarray(seg, dtype=np.int32)
    in_maps = [{"seg": seg[i * TL:(i + 1) * TL], **tables}
               for i in range(_NCORES)]
    res = run_bass_kernel_spmd(nc, in_maps, list(range(_NCORES)))
    out = np.concatenate([res.results[i]["boxes"] for i in range(_NCORES)], axis=0)
    return out.astype(np.float32)


# revision 26
# speedup vs baseline: 1.8632x; 1.0332x over previous
"""Trainium2 Bass kernel for nn_BoxesFromMasks (per-frame segment bounding boxes).

Algorithm (per core, data-parallel over frames, TL=2 frames/core):
  Build per-pixel 64-bit one-hot bitmasks (2 u32 planes) of the instance id via
  the exponent-bit trick (ACT builds the f32 bit pattern of 2^k as an int, a
  second ACT converts value->u32, truncating out-of-range ids to 0):
    lo plane: id s in [0,32)  -> bit (31-s)
    hi plane: id s in [32,64) -> bit (s-32)
  Row masks:  OR-tree each 128-row chunk along columns (DVE), 16-wide leftovers
              folded once at extraction time.
  Col masks:  OR-accumulate chunks into acc[128,2,W]; pre-fold partitions
              128->64; DMA-transpose (u16); OR-tree the 64 contributors.
  Extraction (batched, no DRAM bounce): 16 u16 shift ops expand bits to
  E-tables, constant value-tables select coordinates via i16 mult/add, strided
  tensor_reduce min/max, one 3x128 transpose fold, and negative-stride output
  DMAs undo the bit-order permutation.
"""

import numpy as np

_T, _H, _W, _N = 16, 1024, 2048, 64
_NCORES = 8

_BUILD_CACHE = {}


def _build_program(TL, H, W, split_waits=True, reps=1):
    from contextlib import ExitStack

    import bass_rust
    import concourse.bass as bass
    import concourse.tile as tile
    import concourse.mybir as mybir
    from concourse.alu_op_type import AluOpType as Op

    f32 = mybir.dt.float32
    i32 = mybir.dt.int32
    u32 = mybir.dt.uint32
    u16 = mybir.dt.uint16
    i16 = mybir.dt.int16
    Copy = mybir.ActivationFunctionType.Copy
    X = mybir.AxisListType.X

    P = 128
    CH = H // P                   # row chunks per frame (8)
    KT = 4                        # transpose calls per frame (each 2048 u16 cols)
    MPER = 16                     # 128-col blocks per transpose call
    NSEG = 2                      # seg DMA splits per chunk
    BIG = 32767
    assert TL == 2 and CH == 8 and W == 2048

    # ---- constant value tables (i16) ----
    pp = np.arange(P)
    # Y: value v(p, c) = 128c + p ; table shape [P, 64(pl f j), CH, 2h]
    yv = (128 * np.arange(CH)[None, :] + pp[:, None]).astype(np.int64)   # [P, CH]
    ty_mb = np.broadcast_to((yv - BIG)[:, None, :, None],
                            (P, 64, CH, 2)).astype(np.int16)
    ty_p1 = np.broadcast_to((yv + 1)[:, None, :, None],
                            (P, 64, CH, 2)).astype(np.int16)
    # X: value v(q, klo, m) = klo*1024 + 64m + (q>>1) ; table [P, 64(pl f j), 32(klo m)]
    klo = np.arange(2)
    mm = np.arange(MPER)
    xv = ((klo[:, None] * 1024 + 64 * mm[None, :]).reshape(-1)[None, :]
          + (pp[:, None] // 2)).astype(np.int64)                         # [P, 32]
    tx_mb = np.broadcast_to((xv - BIG)[:, None, :], (P, 64, 32)).astype(np.int16)
    tx_p1 = np.broadcast_to((xv + 1)[:, None, :], (P, 64, 32)).astype(np.int16)

    tables = {"ty_mb": ty_mb, "ty_p1": ty_p1, "tx_mb": tx_mb, "tx_p1": tx_p1}

    nc = bass.Bass()
    seg_in = nc.dram_tensor("seg", [TL, H, W], i32, kind="ExternalInput")
    boxes_out = nc.dram_tensor("boxes", [TL, 64, 4], f32, kind="ExternalOutput")
    d_tabs = {n: nc.dram_tensor(n, list(t.shape), i16, kind="ExternalInput")
              for n, t in tables.items()}

    def dram_ap(t, offset_elems, dims):
        """Manual DRAM AP: dims = [(stride_elems, count), ...]."""
        a2 = t[:].copy()
        a2.offset = offset_elems
        a2.ap = bass_rust.VecI64Pair([[s, n] for s, n in dims])
        return a2

    with tile.TileContext(nc) as tc, ExitStack() as ctx:
        constp = ctx.enter_context(tc.tile_pool(name="consts", bufs=1))
        segp = ctx.enter_context(tc.tile_pool(name="segp", bufs=3))
        ep = ctx.enter_context(tc.tile_pool(name="ep", bufs=3))
        accp = ctx.enter_context(tc.tile_pool(name="accp", bufs=2))
        accTp = ctx.enter_context(tc.tile_pool(name="accTp", bufs=2))
        rmp = ctx.enter_context(tc.tile_pool(name="rmp", bufs=2))
        xp = ctx.enter_context(tc.tile_pool(name="xp", bufs=2))
        smallp = ctx.enter_context(tc.tile_pool(name="smallp", bufs=2))

        c_ty_mb = constp.tile([P, 64, CH, 2], i16)
        c_ty_p1 = constp.tile([P, 64, CH, 2], i16)
        c_tx_mb = constp.tile([P, 64, 32], i16)
        c_tx_p1 = constp.tile([P, 64, 32], i16)
        const_loaded = [False]

        def load_consts():
            if const_loaded[0]:
                return
            const_loaded[0] = True
            for t, n in [(c_ty_mb, "ty_mb"), (c_ty_p1, "ty_p1"),
                         (c_tx_mb, "tx_mb"), (c_tx_p1, "tx_p1")]:
                nc.scalar.dma_start(t[:], d_tabs[n][:])

        for _rep in range(reps):
            # rmask: [p, pl, f, c] u32 (pl-major for contiguous planes)
            rmask = rmp.tile([P, 2, TL, CH], u32, tag="rmask")
            # CMX: [q, pl, f, klo, m] u16 (compacted column masks)
            CMX = xp.tile([P, 2, TL, 2, MPER], u16, tag="cmx")

            # ================= main loop =================
            # seg tiles are created and their loads issued ahead of use so
            # next-frame loads precede this frame's transposes on the queue
            seg_tiles = {}

            def issue_seg(f, c):
                if f >= TL or (f, c) in seg_tiles:
                    return
                s = segp.tile([P, W], i32, tag="seg")
                nseg = 4 if (f == 0 and c == 0) else NSEG
                rows = P // nseg
                for k in range(nseg):
                    nc.sync.dma_start(
                        s[rows * k:rows * (k + 1), :],
                        seg_in[f, c * P + rows * k:c * P + rows * (k + 1), :])
                seg_tiles[(f, c)] = s

            for c2 in range(3):
                issue_seg(0, c2)
            load_consts()

            for f in range(TL):
                acc = accp.tile([P, 2, W], u32)
                prev_u = None
                for c in range(CH):
                    issue_seg(f, c)
                    s = seg_tiles.pop((f, c))

                    e = ep.tile([P, 2, W], i32)
                    # lo: bitpattern of 2^(31-s) = (158-s)<<23 ; hi: 2^(s-32) = (s+95)<<23
                    # first chunk of the kernel: split build/cast into partition
                    # halves so the pipeline fills ~4us sooner
                    halves = ([(0, 64), (64, 128)] if (f == 0 and c == 0)
                              else [(0, 128)])
                    for p0, p1 in halves:
                        nc.scalar.activation(e[p0:p1, 0, :], s[p0:p1], Copy,
                                             bias=1325400064.0, scale=-8388608.0)
                        nc.gpsimd.tensor_scalar(e[p0:p1, 1, :], s[p0:p1],
                                                8388608, 796917760,
                                                Op.mult, Op.add)
                        nc.scalar.activation(e[p0:p1].bitcast(u32),
                                             e[p0:p1].bitcast(f32), Copy)
                    u = e[:].bitcast(u32)  # cast in place

                    # column accumulate (DVE; only DVE has integer bitwise ops)
                    if c == 0:
                        prev_u = u
                    elif c == 1:
                        nc.vector.tensor_tensor(acc[:], u, prev_u, Op.bitwise_or)
                    else:
                        nc.vector.tensor_tensor(acc[:], u, acc[:], Op.bitwise_or)

                    # row masks: single OR-reduce along columns (DVE)
                    nc.vector.tensor_reduce(rmask[:, :, f, c], u, axis=X,
                                            op=Op.bitwise_or)

                # prefetch next frame's first chunks before the transposes so
                # their loads aren't queued behind acc-dependent triggers
                for c2 in range(3):
                    issue_seg(f + 1, c2)

                # ---- frame tail: transpose (u16), fold the 128 contributors
                accT = accTp.tile([P, KT, MPER, P], u16, tag="accT")
                a16 = acc[:].bitcast(u16).rearrange("p a b -> p (a b)")
                for k in range(KT):
                    nc.sync.dma_start(accT[:, k],
                                      a16[:, 2048 * k:2048 * (k + 1)],
                                      transpose=True)
                w = 64
                while w >= 1:
                    nc.vector.tensor_tensor(accT[:, :, :, 0:w],
                                            accT[:, :, :, 0:w],
                                            accT[:, :, :, w:2 * w],
                                            Op.bitwise_or)
                    w //= 2
                # compact: CMX[q, pl, f, klo, m] <- accT[q, (pl,klo), m, 0]
                nc.vector.tensor_copy(
                    CMX[:, :, f, :, :],
                    accT[:, :, :, 0].rearrange("q (pl klo) m -> q pl klo m",
                                               pl=2, klo=2))

            # ================= extraction =================
            # --- Y side ---
            # rm u16 view: [p, pl, f, c, h]  (h = u16 half; bit b32 = 16h + j)
            rm_e = rmask[:].bitcast(u16).rearrange(
                "p pl f (c h) -> p (pl f) c h", c=CH, h=2)
            Ey = xp.tile([P, 2, TL, 16, CH, 2], i16, tag="ey")
            for j in range(16):
                nc.vector.tensor_scalar(
                    Ey[:, :, :, j].rearrange("p pl f c h -> p (pl f) c h").bitcast(u16),
                    rm_e, j, 1, Op.logical_shift_right, Op.bitwise_and)

            ey_flat = Ey[:].rearrange("p pl f j c h -> p (pl f j) c h")
            CY = xp.tile([P, 64, CH, 2], i16, tag="cy")
            S = smallp.tile([P, 384], i16)
            # Y block layout: col = t*128 + h*64 + (pl f j)  (h-major for output DMAs)
            Sy = S[:, 0:256].rearrange("p (t h a) -> p t h a", t=2, h=2, a=64)
            # ymin: min over c of E*(v-BIG)+BIG
            nc.vector.tensor_tensor(CY[:], ey_flat, c_ty_mb[:], Op.mult)
            nc.vector.tensor_scalar(CY[:], CY[:], BIG, None, Op.add)
            nc.vector.tensor_reduce(Sy[:, 0], CY[:].rearrange("p a c h -> p h a c"),
                                    axis=X, op=Op.min)
            # ymax(+1): max over c of E*(v+1)
            nc.vector.tensor_tensor(CY[:], ey_flat, c_ty_p1[:], Op.mult)
            nc.vector.tensor_reduce(Sy[:, 1], CY[:].rearrange("p a c h -> p h a c"),
                                    axis=X, op=Op.max)

            # --- X side ---
            cmx_flat = CMX[:].rearrange("q pl f klo m -> q (pl f) (klo m)")
            Ex = xp.tile([P, 4, 16, 32], i16, tag="ex")
            for j in range(16):
                nc.vector.tensor_scalar(Ex[:, :, j].bitcast(u16), cmx_flat,
                                        j, 1, Op.logical_shift_right, Op.bitwise_and)
            ex_flat = Ex[:].rearrange("q a j km -> q (a j) km")
            CXt = xp.tile([P, 64, 32], i16, tag="cx")
            nc.vector.tensor_tensor(CXt[:], ex_flat, c_tx_mb[:], Op.mult)
            nc.vector.tensor_scalar(CXt[:], CXt[:], BIG, None, Op.add)
            nc.vector.tensor_reduce(S[:, 256:320], CXt[:], axis=X, op=Op.min)
            nc.vector.tensor_tensor(CXt[:], ex_flat, c_tx_p1[:], Op.mult)
            nc.vector.tensor_reduce(S[:, 320:384], CXt[:], axis=X, op=Op.max)

            # --- partition fold: 3 transposes + reduces ---
            ST = smallp.tile([P, 3, 128], i16)
            for t in range(3):
                eng = nc.scalar if t % 2 else nc.sync
                eng.dma_start(ST[:, t], S[:, 128 * t:128 * (t + 1)],
                              transpose=True)
            # Y: rows (pl f j h); reduce over all 128 contributors
            FY = smallp.tile([P, 2], i16)
            nc.vector.tensor_reduce(FY[:, 0:1], ST[:, 0], axis=X, op=Op.min)
            nc.vector.tensor_reduce(FY[:, 1:2], ST[:, 1], axis=X, op=Op.max)
            # X: rows 0:64 = min (pl f j), 64:128 = max; contributors split by
            # parity g = q&1 (halfword h2 = g); output cols = g
            FX = smallp.tile([P, 2], i16)
            nc.vector.tensor_reduce(
                FX[0:64, :], ST[0:64, 2].rearrange("p (x g) -> p g x", g=2),
                axis=X, op=Op.min)
            nc.vector.tensor_reduce(
                FX[64:128, :], ST[64:128, 2].rearrange("p (x g) -> p g x", g=2),
                axis=X, op=Op.max)

            # --- fixups in f32 ---
            # mins: v==BIG (absent) -> 2147483648.0 ; maxes: v-1 == -1 -> -2^31
            BY = smallp.tile([P, 2], f32)
            BX = smallp.tile([P, 2], f32)
            fy = smallp.tile([P, 2], f32)
            fx = smallp.tile([P, 2], f32)
            nc.vector.tensor_copy(BY[:], FY[:])
            nc.vector.tensor_copy(BX[:], FX[:])
            nc.vector.tensor_scalar(BY[:, 1:2], BY[:, 1:2], 1, 0, Op.subtract, Op.add)
            nc.vector.tensor_scalar(BX[64:128, :], BX[64:128, :], 1, 0,
                                    Op.subtract, Op.add)
            nc.vector.tensor_scalar(fy[:, 0:1], BY[:, 0:1], 32767.0, 2147450880.0,
                                    Op.is_equal, Op.mult)
            nc.vector.tensor_scalar(fy[:, 1:2], BY[:, 1:2], -1.0, -2147483647.0,
                                    Op.is_equal, Op.mult)
            nc.vector.tensor_scalar(fx[0:64, :], BX[0:64, :], 32767.0, 2147450880.0,
                                    Op.is_equal, Op.mult)
            nc.vector.tensor_scalar(fx[64:128, :], BX[64:128, :], -1.0, -2147483647.0,
                                    Op.is_equal, Op.mult)
            nc.vector.tensor_tensor(BY[:], BY[:], fy[:], Op.add)
            nc.vector.tensor_tensor(BX[:], BX[:], fx[:], Op.add)

            # --- output DMAs ---
            # boxes[f, n, k]: k: 0 xmin, 1 ymin, 2 xmax, 3 ymax
            # Y rows p = h*64 + (pl*2+f)*16 + j ; n: pl=0: 31-16h-j, pl=1: 32+16h+j
            outn = [0]

            def out_dma(dst, src):
                eng = nc.scalar if outn[0] % 2 else nc.sync
                outn[0] += 1
                eng.dma_start(dst, src)

            for col, k in [(0, 1), (1, 3)]:
                for h in range(2):
                    base = h * 64
                    out_dma(dram_ap(boxes_out, k + 4 * (31 - 16 * h),
                                    [(256, TL), (-4, 16)]),
                            BY[base:base + 32, col:col + 1])
                    out_dma(dram_ap(boxes_out, k + 4 * (32 + 16 * h),
                                    [(256, TL), (4, 16)]),
                            BY[base + 32:base + 64, col:col + 1])
            # X rows p = base + (pl*2+f)*16+j ; n: pl=0: 31-16g-j, pl=1: 32+16g+j
            for base, k in [(0, 0), (64, 2)]:
                for g in range(2):
                    out_dma(dram_ap(boxes_out, k + 4 * (31 - 16 * g),
                                    [(256, TL), (-4, 16)]),
                            BX[base:base + 32, g:g + 1])
                    out_dma(dram_ap(boxes_out, k + 4 * (32 + 16 * g),
                                    [(256, TL), (4, 16)]),
                            BX[base + 32:base + 64, g:g + 1])

    nc.finalize()
    if split_waits:
        _split_excess_waits(nc, mybir)
    return nc, tables


def _split_excess_waits(nc, mybir):
    """Hoist extra sem waits onto preceding NoOps.

    This walrus build rejects instructions carrying more sync-wait
    conditions than their ISA encoding holds (1 for TPB_CTRL ops and for
    Pool/core_v2 compute ops; 2 elsewhere, conservatively). Semantics are
    identical with the waits split onto dedicated NoOps just before the
    instruction.
    """
    n_split = 0
    for f in nc.m.functions:
        for bb in f.blocks:
            newl = []
            for ins in bb.instructions:
                si = ins.sync_info
                max_waits = 1
                if si and si.on_wait and len(si.on_wait) > max_waits:
                    waits = list(si.on_wait)
                    for j, w in enumerate(waits[max_waits:]):
                        nop = mybir.InstNoOp(
                            name=f"{ins.name}-w{j}", ins=[], outs=[],
                            engine=ins.engine,
                            sync_info=mybir.SyncInfo(on_wait=[w], on_update=[]))
                        newl.append(nop)
                        n_split += 1
                    ins.sync_info = mybir.SyncInfo(on_wait=waits[:max_waits],
                                                   on_update=si.on_update)
                newl.append(ins)
            bb.instructions = newl
    return n_split


def _get_program(TL, H, W, reps=1):
    key = (TL, H, W, reps)
    if key not in _BUILD_CACHE:
        _BUILD_CACHE[key] = _build_program(TL, H, W, reps=reps)
    return _BUILD_CACHE[key]


def kernel(segmentation, num_instances=None, **_ignored):
    from concourse.bass_utils import run_bass_kernel_spmd

    seg = np.asarray(segmentation)
    T, H, W = seg.shape
    assert T % _NCORES == 0
    TL = T // _NCORES
    nc, tables = _get_program(TL, H, W)

    seg = np.ascontiguousarray(seg, dtype=np.int32)
    in_maps = [{"seg": seg[i * TL:(i + 1) * TL], **tables}
               for i in range(_NCORES)]
    res = run_bass_kernel_spmd(nc, in_maps, list(range(_NCORES)))
    out = np.concatenate([res.results[i]["boxes"] for i in range(_NCORES)], axis=0)
    return out.astype(np.float32)
